# revision 58
# baseline (speedup 1.0000x reference)
"""MoE routing kernel for Trainium2 (8 NeuronCores, paired-expert F-sharding).

Sharding strategy:
  - The host computes the gate (same math as the reference, on CPU jax so
    tie-breaking matches bit-for-bit) and pairs experts heavy-with-light
    by routed load (e.g. 592+453). Core pair (2g, 2g+1) owns expert pair
    g: each core of the pair processes ALL of both experts' routed tokens
    but only HALF of the F channels (F-sharding), so per-core compute is
    uniform across the chip regardless of routing imbalance.
  - The shared expert is token-sharded across the 4 groups (512 tokens
    each) and F-sharded across the pair, same emitter.
  - Every core therefore runs 3 FFN batches: expert-A tokens (cap 656),
    expert-B tokens (cap 528), shared slice (512) — 1696 columns of
    half-F work ≈ the ideal 64.4 GFLOP / 8 cores.
  - All matmul operands are bf16 (fp32 PSUM accumulate): halves HBM
    traffic; end-to-end quantization error ~5e-3 absmax-rel.
  - Unshard on host: partial outputs of the two F-halves (bf16) add;
    shared slices concatenate; routed outputs scatter-add by token index.
    Combine weights are applied on-device for b_/s_ (ACT/DVE scale); the
    a_ batch stores transposed raw yT[D, cap] (stage-B computes cap=648
    token columns instead of 768 padded rows) and the host applies cw
    plus the down-proj biases during combination.

Schedule/overlap techniques (all measured on HW):
  - fp32 warmup matmuls on memset data release the PE's HAM clock gate
    (1.2 -> 2.4 GHz) before the first real matmul and bridge the
    opening-DMA latency window; bf16 warmups do NOT trip the HAM.
  - The opening loads only chunk0's x columns per-dk in consumption
    order; the fi0 loop is dk-outer into 4 parallel PSUM banks so each
    arriving x tile immediately feeds 4 matmuls (just-in-time start).
  - The 16 DMA rings fair-share among outstanding transfers, so all
    mid-kernel loads are paced: issued on the ACT queue at emission
    points behind compute ops, keeping <=2 transfers in flight and
    arrival ~= need order. Result: a gap-free warm matmul stream within
    ~1.5% of the bf16 1-column/cycle architectural floor.

A dense all-on-device fallback (every core processes all tokens through
its expert, masked by gate weights computed on-device) is kept for the
(never observed) case that a pair's load exceeds capacity.
"""

import numpy as np
from contextlib import ExitStack

import ml_dtypes

import concourse.bass as bass
import concourse.mybir as mybir
import concourse.tile as tile
from concourse import bacc
from concourse.bass_utils import run_bass_kernel_spmd

# Problem dims (hardcoded per contract)
E = 8
D = 1024
F = 1024
T = 2048          # B*S = 2*1024
P = 128
DK = D // P       # 8 k-chunks over D
FH = F // 2       # 512 channels per core (F-shard half)
FI = FH // P      # 4 f-chunks per branch per core
ALPHA = 1.702
LIMIT = 7.0
NCORES = 8
NGROUPS = 4
CAP_A = 648       # cap for the heavier expert of each pair (max load 642)
CAP_B = 512       # cap for the lighter expert (max load 508; 4 full tiles)
CAP_S = T // NGROUPS  # shared-expert tokens per group

F32 = mybir.dt.float32
F32R = mybir.dt.float32r
BF16 = mybir.dt.bfloat16
AF = mybir.ActivationFunctionType
OP = mybir.AluOpType

BF = ml_dtypes.bfloat16


FAST_TEARDOWN = True


class _LeanTC(tile.TileContext):
    """TileContext with a lighter end-of-program teardown.

    The stock teardown (drain + barrier + gpsimd dma_reset + sem_clear +
    barrier) measures ~6us of pure serialized tail on the slow gpsimd
    sequencer AFTER the last data movement. The dma_reset (a Pool-engine
    InstDrain over the kernel's semaphore range) is only needed so a
    subsequent execution of the same loaded NEFF sees quiesced DMA
    state; we run each program once per process, so skip it and keep
    only the semaphore clear."""

    def _drain_and_barrier(self, tick_clock, wait_clock):
        from concourse.vector_clock import ScopedClock
        from concourse.bass import compact_to_ranges
        nc = self.nc
        drain_inst = nc.sync.drain()
        wait_clock.add_sem_waits(
            drain_inst.ins, ScopedClock({None: tick_clock.global_clock}))
        nc.all_engine_barrier()
        popped = nc._tile_sem_poison_stack.pop()
        assert popped is self._sem_poison
        sems = list(self.sems.allocated().values())
        sem_nums = [s.num if hasattr(s, "num") else int(s) for s in sems]
        for rng in compact_to_ranges(sem_nums):
            assert nc._state.free_isdisjoint(rng)
            nc.gpsimd.sem_clear(rng)
        nc._state.prepend_free_semaphores(sem_nums)
        for poison_set in nc._tile_sem_poison_stack:
            poison_set.update(sem_nums)
        nc.all_engine_barrier()


def _chunks(n):
    # near-equal chunks <= 512 (a tiny tail chunk wastes the ~60-cycle
    # matmul issue floor, so balance instead: 528 -> 264+264)
    k = -(-n // 512)
    base = n // k
    rem = n - base * k
    out = []
    o = 0
    for i in range(k):
        s = base + (1 if i < rem else 0)
        out.append((o, s))
        o += s
    return out


# ---------------------------------------------------------------------------
# generic FFN-batch emitter (half-F):
#   out[cap, D] = cw * (swiglu((xT@w1+b1)*(xT@w3+b3)) @ (w2T/alpha) + b2)
# where swiglu' returns alpha*a (the 1/alpha is folded into w2T on host).
# ---------------------------------------------------------------------------

def _emit_warmup(tc, pools, n_mms=11):
    """Dummy matmuls on memset data, issued before any input-dependent
    work. They run during the ~5us opening-DMA latency window, so the
    PE's HAM clock gate (4096-cycle activity window, ~3.4us) releases
    to 8/8 before the first real matmul — otherwise the opening ~3.4us
    of real matmuls run at 1.2 GHz instead of 2.4. Counterintuitively
    the tiles MUST be fp32: measured on HW, a stream of 28 bf16 N=128
    matmuls (~50% PE-array duty due to interleaved LDWEIGHTS) never
    tripped the HAM activity window, while fp32 (4 cycles/row, ~80%
    duty) trips it ~2.8us after the first warmup MM. 8 fp32 MMs =
    ~3.4us, ending about when the first real weight/x tiles land."""
    nc = tc.nc
    wA, w2p, apool, hpool, outp, psA, psB = pools
    wz = hpool.tile([P, P], F32, tag="wz")
    xz = hpool.tile([P, P], F32, tag="xz")
    nc.gpsimd.memset(wz[:], 0.0)
    nc.gpsimd.memset(xz[:], 0.0)
    for _ in range(n_mms):
        ps = psA.tile([P, 512], F32, tag="pA")
        nc.tensor.matmul(ps[:, :P], wz[:], xz[:], start=True, stop=True)


WSETS = ("w1g", "w1l", "w3g", "w3l")


def _mk_loader(tc, pools, aps):
    """Tile-allocating DMA closures + per-batch state.

    Key scheduling fact (measured): the 16 DMA rings FAIR-SHARE among
    all outstanding transfers, so a prefetch issued early delays the
    first-needed transfer proportionally. Mid-kernel loads are therefore
    issued on the ACT (scalar) queue at chosen emission points between
    compute ops — the in-order sequencer only reaches the dma_start
    after the preceding compute retires, pacing issues to ~need order
    with at most ~2 transfers in flight."""
    nc = tc.nc
    wA, w2p, apool, hpool, outp, psA, psB = pools
    state = {}

    def st(pref):
        return state.setdefault(pref, {"wcs": {}})

    def wcat(pref, fi, eng=None):
        wc = wA.tile([P, 4, DK, P], BF16, tag="wcat")
        (eng or nc.scalar).dma_start(wc[:], aps[pref + "wcat"][fi])

        def mk(j):
            return lambda dk: wc[:, j, dk, :]

        st(pref)["wcs"][fi] = {nm: mk(j) for j, nm in enumerate(WSETS)}

    def xe(pref, cap, eng=None):
        t = apool.tile([P, DK // 2, cap], BF16, tag=pref + "xe")
        (eng or nc.scalar).dma_start(t[:], aps[pref + "xe"][:])
        st(pref)["xe"] = t

    def xo(pref, cap, eng=None):
        t = apool.tile([P, DK // 2, cap], BF16, tag=pref + "xo")
        (eng or nc.scalar).dma_start(t[:], aps[pref + "xo"][:])
        s = st(pref)
        s["xo"] = t
        xet = s["xe"]

        def xat(dk, to, ts):
            return (xet if dk % 2 == 0 else t)[:, dk // 2, to:to + ts]

        s["xat"] = xat

    def ballcw(pref, cap, eng=None):
        ntt = (cap + P - 1) // P
        t = apool.tile([P, 4 * FI + ntt], F32, tag=pref + "ballcw")
        (eng or nc.scalar).dma_start(t[:], aps[pref + "ballcw"][:])
        st(pref)["ballcw"] = t

    def w2T(pref, eng=None):
        # a_ uses the transposed stage-B layout (same byte size)
        shape = [P, FI, DK, P] if pref == "a_" else [P, FI, D]
        t = w2p.tile(shape, BF16, tag="w2t")
        (eng or nc.scalar).dma_start(t[:], aps[pref + "w2T"][:])
        st(pref)["w2t"] = t

    return state, dict(wcat=wcat, xe=xe, xo=xo, ballcw=ballcw, w2T=w2T)


def _emit_opening(tc, pools, aps, state, load, cap):
    """Opening flood for the first batch: per-dk chunk0-only x tiles +
    per-wset fi0 weight tiles interleaved across both HWDGE queues in
    CONSUMPTION order — the fi0-chunk0 loop is dk-outer, so each
    arriving x tile immediately enables 4 matmuls and the PE streams
    just-in-time behind the DMAs. Only chunk0's x columns ride the
    critical window (~1.7MB instead of 2.3MB); chunk1's columns follow
    as two merged transfers and land during chunk0 compute. The fi1
    weights are NOT issued here — the rings fair-share among all
    outstanding transfers, so any early prefetch delays the whole
    opening; fi1 is paced after chunk0's chain instead. The gpsimd
    queue is never used: it is a software-DGE path that trickles data
    and (measured) starves everything sharing the rings."""
    nc = tc.nc
    wA, w2p, apool, hpool, outp, psA, psB = pools
    s = state.setdefault("a_", {"wcs": {}})
    (c0o, c0n), (c1o, c1n) = _chunks(cap)

    # fi0 weights split into dk0-3 / dk4-7 half-tiles: only the first
    # halves (~0.5MB) gate the first matmul; the second halves arrive
    # while the dk-outer loop is consuming dk0-3.
    weng = {"w1g": nc.sync, "w1l": nc.sync,
            "w3g": nc.scalar, "w3l": nc.scalar}
    wh = {}
    for nm in ("w1g", "w3g", "w1l", "w3l"):
        t = wA.tile([P, DK // 2, P], BF16, tag="w0" + nm + "h0")
        weng[nm].dma_start(t[:], aps["a_" + nm + "h0"][:])
        wh[nm, 0] = t

    xc0 = []
    for dk in range(DK):
        t = apool.tile([P, c0n], BF16, tag=f"a_x{dk}")
        eng = nc.sync if dk % 2 == 0 else nc.scalar
        eng.dma_start(
            t[:], aps["a_xT"][dk * P:(dk + 1) * P, c0o:c0o + c0n])
        xc0.append(t)

    for nm in ("w1g", "w3g", "w1l", "w3l"):
        t = wA.tile([P, DK // 2, P], BF16, tag="w0" + nm + "h1")
        weng[nm].dma_start(t[:], aps["a_" + nm + "h1"][:])
        wh[nm, 1] = t

    def mkw(nm):
        return lambda dk: wh[nm, dk // (DK // 2)][:, dk % (DK // 2), :]

    s["wcs"][0] = {nm: mkw(nm) for nm in WSETS}

    xc1e = apool.tile([P, DK // 2, c1n], BF16, tag="a_xc1e")
    nc.sync.dma_start(xc1e[:], aps["a_xc1e"][:])
    xc1o = apool.tile([P, DK // 2, c1n], BF16, tag="a_xc1o")
    nc.scalar.dma_start(xc1o[:], aps["a_xc1o"][:])

    def xat(dk, to, ts):
        if to == c0o:
            return xc0[dk][:, :ts]
        return (xc1e if dk % 2 == 0 else xc1o)[:, dk // 2, :ts]

    s["xat"] = xat
    load["ballcw"]("a_", cap, nc.scalar)


def _emit_stage_a(tc, pools, pref, aps, cap, tiles, first=False,
                  pace=None):
    """pace: {position: [thunk]} — thunks (paced ACT-queue dma_starts)
    run after the chunk at that position in the (fi, chunk) chain."""
    nc = tc.nc
    wA, w2p, apool, hpool, outp, psA, psB = pools
    wcs, xat = tiles["wcs"], tiles["xat"]
    ball = tiles["ballcw"]
    pace = pace or {}
    ntt = (cap + P - 1) // P

    def swiglu_chain(pg1, pg3, pl1, pl3, ts, biases, out_ap):
        bc1g, bc1l, bc3g, bc3l = biases
        t1 = hpool.tile([P, 512], F32, tag="tcp")
        nc.scalar.activation(t1[:, :ts], pg1[:, :ts], AF.Identity,
                             bias=bc1g)
        hg = hpool.tile([P, 512], F32, tag="hh")
        nc.vector.scalar_tensor_tensor(
            out=hg[:, :ts], in0=pg3[:, :ts], scalar=bc3g, in1=t1[:, :ts],
            op0=OP.add, op1=OP.mult)
        nc.vector.tensor_scalar_min(hg[:, :ts], hg[:, :ts], LIMIT)
        gs = hpool.tile([P, 512], F32, tag="gs")
        nc.scalar.activation(gs[:, :ts], hg[:, :ts], AF.Silu, scale=ALPHA)
        t2 = hpool.tile([P, 512], F32, tag="tcp")
        nc.scalar.activation(t2[:, :ts], pl1[:, :ts], AF.Identity,
                             bias=bc1l)
        hl = hpool.tile([P, 512], F32, tag="hh")
        nc.vector.scalar_tensor_tensor(
            out=hl[:, :ts], in0=pl3[:, :ts], scalar=bc3l, in1=t2[:, :ts],
            op0=OP.add, op1=OP.mult)
        nc.vector.tensor_scalar(
            out=hl[:, :ts], in0=hl[:, :ts], scalar1=LIMIT, scalar2=-LIMIT,
            op0=OP.min, op1=OP.max)
        # a = (hl + 1) * gs   (the 1/alpha lives in w2T)
        nc.vector.scalar_tensor_tensor(
            out=out_ap, in0=hl[:, :ts], scalar=1.0,
            in1=gs[:, :ts], op0=OP.add, op1=OP.mult)

    pos = 0

    def run_pace():
        for thunk in pace.get(pos, ()):
            thunk()

    atiles = []
    for fi in range(FI):
        at = apool.tile([P, cap], BF16, tag=f"{pref}a{fi}")
        atiles.append(at)
        wt = wcs[fi]
        biases = (ball[:, 0 * FI + fi:0 * FI + fi + 1],
                  ball[:, 1 * FI + fi:1 * FI + fi + 1],
                  ball[:, 2 * FI + fi:2 * FI + fi + 1],
                  ball[:, 3 * FI + fi:3 * FI + fi + 1])

        chunks = _chunks(cap)
        if fi == 0 and first:
            # JIT opening: dk-outer over 4 parallel PSUM accumulations so
            # each x tile feeds the PE the moment its DMA lands.
            (to, ts) = chunks[0]
            tsl = slice(to, to + ts)
            pg1 = psA.tile([P, 512], F32, tag="pA")
            pg3 = psB.tile([P, 512], F32, tag="pB2")
            pl1 = psA.tile([P, 512], F32, tag="pB")
            pl3 = psB.tile([P, 512], F32, tag="pB2")
            pmap = {"w1g": pg1, "w3g": pg3, "w1l": pl1, "w3l": pl3}
            for dk in range(DK):
                for nm in ("w1g", "w3g", "w1l", "w3l"):
                    nc.tensor.matmul(
                        pmap[nm][:, :ts], (wt[nm](dk)),
                        (xat(dk, to, ts)),
                        start=(dk == 0), stop=(dk == DK - 1))
            swiglu_chain(pg1, pg3, pl1, pl3, ts, biases, at[:, tsl])
            run_pace()
            pos += 1
            chunks = chunks[1:]

        for (to, ts) in chunks:
            tsl = slice(to, to + ts)

            def hpsum(wtile, ptag):
                ps = psA.tile([P, 512], F32, tag=ptag)
                for dk in range(DK):
                    nc.tensor.matmul(
                        ps[:, :ts], (wtile(dk)),
                        (xat(dk, to, ts)),
                        start=(dk == 0), stop=(dk == DK - 1))
                return ps

            pg1 = hpsum(wt["w1g"], "pA")
            t1 = hpool.tile([P, 512], F32, tag="tcp")
            nc.scalar.activation(t1[:, :ts], pg1[:, :ts], AF.Identity,
                                 bias=biases[0])
            pg3 = hpsum(wt["w3g"], "pB")
            hg = hpool.tile([P, 512], F32, tag="hh")
            nc.vector.scalar_tensor_tensor(
                out=hg[:, :ts], in0=pg3[:, :ts], scalar=biases[2],
                in1=t1[:, :ts], op0=OP.add, op1=OP.mult)
            nc.vector.tensor_scalar_min(hg[:, :ts], hg[:, :ts], LIMIT)
            gs = hpool.tile([P, 512], F32, tag="gs")
            nc.scalar.activation(gs[:, :ts], hg[:, :ts], AF.Silu, scale=ALPHA)

            pl1 = hpsum(wt["w1l"], "pA")
            t2 = hpool.tile([P, 512], F32, tag="tcp")
            nc.scalar.activation(t2[:, :ts], pl1[:, :ts], AF.Identity,
                                 bias=biases[1])
            pl3 = hpsum(wt["w3l"], "pB")
            hl = hpool.tile([P, 512], F32, tag="hh")
            nc.vector.scalar_tensor_tensor(
                out=hl[:, :ts], in0=pl3[:, :ts], scalar=biases[3],
                in1=t2[:, :ts], op0=OP.add, op1=OP.mult)
            nc.vector.tensor_scalar(
                out=hl[:, :ts], in0=hl[:, :ts], scalar1=LIMIT, scalar2=-LIMIT,
                op0=OP.min, op1=OP.max)
            # a = (hl + 1) * gs   (the 1/alpha lives in w2T)
            nc.vector.scalar_tensor_tensor(
                out=atiles[fi][:, tsl], in0=hl[:, :ts], scalar=1.0,
                in1=gs[:, :ts], op0=OP.add, op1=OP.mult)
            run_pace()
            pos += 1
    tiles["atiles"] = atiles


def _emit_stage_bT(tc, pools, pref, aps, cap, tiles):
    """Transposed stage B for the capacity-padded a_ batch: computes
    yT[D, cap] = (w2T)ᵀ-blocks @ a instead of aᵀ @ w2T. PE cost scales
    with the token-column count (cap=648) rather than the padded
    ceil(cap/128)*128 = 768, saving ~1.5us. The combine weight cannot
    be applied per-COLUMN on device, so for this batch raw yT partials
    are stored and the host applies cw during the scatter-add."""
    nc = tc.nc
    wA, w2p, apool, hpool, outp, psA, psB = pools
    atiles, w2d = tiles["atiles"], tiles["w2t"]
    g = 0
    for db in range(DK):
        ot = outp.tile([P, cap], BF16, tag="otT")
        for (to, ts) in _chunks(cap):
            pB = psB.tile([P, 512], F32, tag="pB2")
            for fi in range(FI):
                nc.tensor.matmul(
                    pB[:, :ts], (w2d[:, fi, db, :]),
                    (atiles[fi][:, to:to + ts]),
                    start=(fi == 0), stop=(fi == FI - 1))
            if g % 2 == 0:
                nc.scalar.activation(ot[:, to:to + ts], pB[:, :ts],
                                     AF.Identity)
            else:
                nc.vector.tensor_scalar(
                    out=ot[:, to:to + ts], in0=pB[:, :ts],
                    scalar1=0.0, scalar2=0.0, op0=OP.add, op1=OP.add)
            g += 1
        nc.scalar.dma_start(
            aps[pref + "out"][db * P:(db + 1) * P, :], ot[:])


def _emit_stage_b(tc, pools, pref, aps, cap, tiles, last=False):
    # b2 is added on the host; DVE applies the combine weight and writes
    # bf16 partials; the next batch's loads were already queued before
    # these stores, so sync-queue ordering cannot starve the PE. On the
    # final batch the trailing groups drain on ACT as well so the
    # end-of-program backlog clears twice as fast.
    nc = tc.nc
    wA, w2p, apool, hpool, outp, psA, psB = pools
    atiles, w2t = tiles["atiles"], tiles["w2t"]
    ballcw = tiles["ballcw"]
    CWO = 4 * FI
    ntt = (cap + P - 1) // P
    g = 0
    for tp in range(ntt):
        tn = min(P, cap - tp * P)
        tsl = slice(tp * P, tp * P + tn)
        ot = outp.tile([P, D], BF16, tag="ot")
        for dch in range(D // 512):
            dsl = slice(dch * 512, (dch + 1) * 512)
            pB = psB.tile([P, 512], F32, tag="pB2")
            for fi in range(FI):
                nc.tensor.matmul(
                    pB[:tn, :], (atiles[fi][:, tsl]), (w2t[:, fi, dsl]),
                    start=(fi == 0), stop=(fi == FI - 1))
            if last and g % 2 == 0:
                nc.scalar.activation(ot[:tn, dsl], pB[:tn, :], AF.Identity,
                                     scale=ballcw[:tn, CWO + tp:CWO + tp + 1])
            else:
                nc.vector.tensor_scalar_mul(
                    ot[:tn, dsl], pB[:tn, :],
                    ballcw[:tn, CWO + tp:CWO + tp + 1])
            g += 1
            if last and tp == ntt - 1:
                # split the final tile's store per 512-col half so the
                # first half streams out while the second is computed
                nc.scalar.dma_start(
                    aps[pref + "out"][tsl, dsl], ot[:tn, dsl])
        if not (last and tp == ntt - 1):
            # stores ride the scalar queue: the sync queue carries the
            # next batch's large weight/x prefetches; ordering behind
            # them would delay the output drain.
            nc.scalar.dma_start(aps[pref + "out"][tsl, :], ot[:tn, :])


def _build_sparse():
    nc = bacc.Bacc(
        "TRN2", target_bir_lowering=False, debug=False, num_devices=NCORES
    )
    aps = {}

    def inp(name, shape, dt=F32):
        aps[name] = nc.dram_tensor(name, shape, dt, kind="ExternalInput").ap()

    for pref, cap in (("a_", CAP_A), ("b_", CAP_B), ("s_", CAP_S)):
        ntt = (cap + P - 1) // P
        if pref == "a_":
            inp(pref + "xT", [D, cap], BF16)
            c1n = _chunks(cap)[1][1]
            inp(pref + "xc1e", [P, DK // 2, c1n], BF16)
            inp(pref + "xc1o", [P, DK // 2, c1n], BF16)
        else:
            inp(pref + "xe", [P, DK // 2, cap], BF16)
            inp(pref + "xo", [P, DK // 2, cap], BF16)
        inp(pref + "wcat", [FI, P, 4, DK, P], BF16)
        inp(pref + "ballcw", [P, 4 * FI + ntt])
        if pref == "a_":
            inp(pref + "w2T", [P, FI, DK, P], BF16)
            aps[pref + "out"] = nc.dram_tensor(
                pref + "out", [D, cap], BF16, kind="ExternalOutput").ap()
        else:
            inp(pref + "w2T", [P, FI, D], BF16)
            aps[pref + "out"] = nc.dram_tensor(
                pref + "out", [cap, D], BF16, kind="ExternalOutput").ap()
    for n in ("w1g", "w1l", "w3g", "w3l"):
        inp("a_" + n + "h0", [P, DK // 2, P], BF16)
        inp("a_" + n + "h1", [P, DK // 2, P], BF16)

    tc_cls = _LeanTC if FAST_TEARDOWN else tile.TileContext
    with tc_cls(nc) as tc:
        with ExitStack() as ctx:
            wA = ctx.enter_context(tc.tile_pool(name="wA", bufs=3))
            w2p = ctx.enter_context(tc.tile_pool(name="w2p", bufs=2))
            apool = ctx.enter_context(tc.tile_pool(name="apool", bufs=1))
            hpool = ctx.enter_context(tc.tile_pool(name="hpool", bufs=4))
            outp = ctx.enter_context(tc.tile_pool(name="outp", bufs=6))
            psA = ctx.enter_context(
                tc.tile_pool(name="psA", bufs=2, space="PSUM"))
            psB = ctx.enter_context(
                tc.tile_pool(name="psB", bufs=4, space="PSUM"))
            pools = (wA, w2p, apool, hpool, outp, psA, psB)
            batches = (("a_", CAP_A), ("b_", CAP_B), ("s_", CAP_S))
            state, load = _mk_loader(tc, pools, aps)
            _emit_warmup(tc, pools)
            _emit_opening(tc, pools, aps, state, load, CAP_A)

            def L(kind, pref, *a):
                return lambda: load[kind](pref, *a)

            # Paced-prefetch schedule: position -> loads issued right
            # after that (fi, chunk) completes emission on the ACT queue.
            # Each load lands ~5-20us before its consumer with <=2
            # transfers in flight (the rings fair-share, so flooding
            # them delays the first-needed transfer).
            paces = {
                "a_": {
                    0: [L("wcat", "a_", 1)],
                    1: [L("wcat", "a_", 2)],
                    2: [L("wcat", "a_", 3)],
                    3: [L("w2T", "a_")],
                    4: [L("wcat", "b_", 0), L("xe", "b_", CAP_B)],
                    5: [L("xo", "b_", CAP_B), L("ballcw", "b_", CAP_B)],
                    6: [L("wcat", "b_", 1)],
                },
                "b_": {
                    0: [L("wcat", "b_", 2)],
                    1: [L("wcat", "b_", 3), L("w2T", "b_")],
                    2: [L("wcat", "s_", 0), L("xe", "s_", CAP_S)],
                    3: [L("xo", "s_", CAP_S), L("ballcw", "s_", CAP_S),
                        L("wcat", "s_", 1)],
                },
                "s_": {
                    0: [L("wcat", "s_", 2)],
                    1: [L("wcat", "s_", 3), L("w2T", "s_")],
                },
            }
            for i, (pref, cap) in enumerate(batches):
                _emit_stage_a(tc, pools, pref, aps, cap, state[pref],
                              first=(i == 0), pace=paces[pref])
                if pref == "a_":
                    _emit_stage_bT(tc, pools, pref, aps, cap, state[pref])
                else:
                    _emit_stage_b(tc, pools, pref, aps, cap, state[pref],
                                  last=(i == len(batches) - 1))
    nc.compile()
    return nc


# ---------------------------------------------------------------------------
# host-side prep
# ---------------------------------------------------------------------------

def _warr(w):      # [FH, D] -> [FI, P, DK, P] stage-A stationary layout
    return np.ascontiguousarray(
        w.T.reshape(DK, P, FI, P).transpose(2, 1, 0, 3))


def _bcol(b):      # [FH] -> [P, FI]
    return np.ascontiguousarray(b.reshape(FI, P).T)


def _gate(x, gate_w, gate_b):
    """Replicate the reference gate on CPU jax (bit-identical math)."""
    import jax
    import jax.numpy as jnp
    cpu = jax.devices("cpu")[0]
    with jax.default_device(cpu):
        xt = jnp.asarray(np.asarray(x, np.float32).reshape(T, D))
        logits = xt @ jnp.asarray(np.asarray(gate_w, np.float32)).T
        scores = jax.nn.softmax(logits.astype(jnp.float32), axis=-1)
        biased = scores + jnp.asarray(
            np.asarray(gate_b, np.float32)).astype(jnp.float32)
        idx = jax.lax.top_k(biased, 2)[1]
        weights = jnp.take_along_axis(scores, idx, axis=-1)
        return np.asarray(idx), np.asarray(weights)


def _prep_sparse(x, gate_w, gate_b, w1, b1, w3, b3, w2, b2,
                 sw1, sb1, sw3, sb3, sw2, sb2):
    f32 = np.float32
    xt = np.asarray(x, f32).reshape(T, D)
    xTq = np.ascontiguousarray(xt.T.astype(BF))     # [D, T] bf16

    idx, wts = _gate(x, gate_w, gate_b)             # [T, 2], [T, 2]
    toks = [[] for _ in range(E)]
    cws = [[] for _ in range(E)]
    for k in range(2):
        for t in range(T):
            e = int(idx[t, k])
            toks[e].append(t)
            cws[e].append(wts[t, k])
    counts = np.array([len(v) for v in toks])

    # pair heavy-with-light
    order = np.argsort(counts, kind='stable')
    eAs = [int(order[7 - g]) for g in range(NGROUPS)]   # heavier experts
    eBs = [int(order[g]) for g in range(NGROUPS)]       # lighter experts
    if counts[eAs].max() > CAP_A or counts[eBs].max() > CAP_B:
        return None, None, None  # fall back to dense

    def halves(w1e, b1e, w3e, b3e, w2e, b2e):
        """Per-F-half weight dict pieces for one expert's matrices."""
        w1e, w3e = np.asarray(w1e, f32), np.asarray(w3e, f32)
        b1e, b3e = np.asarray(b1e, f32), np.asarray(b3e, f32)
        w2e, b2e = np.asarray(w2e, f32), np.asarray(b2e, f32)
        out = []
        for h in range(2):
            fsl = slice(h * FH, (h + 1) * FH)
            parts = {
                "w1g": _warr(w1e[0::2][fsl].astype(BF)),
                "w1l": _warr(w1e[1::2][fsl].astype(BF)),
                "w3g": _warr(w3e[0::2][fsl].astype(BF)),
                "w3l": _warr(w3e[1::2][fsl].astype(BF)),
            }
            w2T = np.ascontiguousarray(
                (w2e.T[fsl] * (1.0 / ALPHA)).astype(BF))
            out.append({
                "wcat": np.ascontiguousarray(np.stack(
                    [parts[nm] for nm in WSETS], axis=2)),
                "w1g0": parts["w1g"][0], "w1l0": parts["w1l"][0],
                "w3g0": parts["w3g"][0], "w3l0": parts["w3l"][0],
                "ball": np.ascontiguousarray(np.concatenate(
                    [_bcol(b1e[0::2][fsl]), _bcol(b1e[1::2][fsl]),
                     _bcol(b3e[0::2][fsl]), _bcol(b3e[1::2][fsl])], axis=1)),
                "w2T": np.ascontiguousarray(
                    w2T.reshape(FI, P, D).transpose(1, 0, 2)),
                "w2TT": np.ascontiguousarray(
                    w2T.reshape(FI, P, DK, P).transpose(1, 0, 2, 3)),
            })
        return out

    def gather(tl, cwv, cap):
        n = len(tl)
        tpad = np.zeros(cap, np.int64)
        tpad[:n] = tl
        cpad = np.zeros(((cap + P - 1) // P) * P, f32)
        cpad[:n] = cwv
        xg = np.ascontiguousarray(xTq[:, tpad])
        cwcol = np.ascontiguousarray(
            cpad.reshape(-1, P).T)
        return xg, cwcol

    sh_halves = halves(sw1, sb1, sw3, sb3, sw2, sb2)

    def assemble(m, pref, hv, xpack, cwcol):
        m[pref + "wcat"] = hv["wcat"]
        m[pref + "w2T"] = hv["w2TT"] if pref == "a_" else hv["w2T"]
        m[pref + "ballcw"] = np.ascontiguousarray(
            np.concatenate([hv["ball"], cwcol], axis=1))
        if pref == "a_":
            m["a_xT"] = xpack
            c1o, c1n = _chunks(CAP_A)[1]
            x3 = xpack[:, c1o:c1o + c1n].reshape(DK, P, c1n)
            m["a_xc1e"] = np.ascontiguousarray(x3[0::2].transpose(1, 0, 2))
            m["a_xc1o"] = np.ascontiguousarray(x3[1::2].transpose(1, 0, 2))
            for nm in WSETS:
                w0 = hv[nm + "0"]
                m["a_" + nm + "h0"] = np.ascontiguousarray(
                    w0[:, :DK // 2])
                m["a_" + nm + "h1"] = np.ascontiguousarray(
                    w0[:, DK // 2:])
        else:
            m[pref + "xe"], m[pref + "xo"] = xpack

    def eo_split(xg, cap):
        x3 = xg.reshape(DK, P, cap)
        return (np.ascontiguousarray(x3[0::2].transpose(1, 0, 2)),
                np.ascontiguousarray(x3[1::2].transpose(1, 0, 2)))

    in_maps = [dict() for _ in range(NCORES)]
    meta = []
    for g in range(NGROUPS):
        eA, eB = eAs[g], eBs[g]
        xgA, cwA = gather(toks[eA], cws[eA], CAP_A)
        xgB, cwB = gather(toks[eB], cws[eB], CAP_B)
        ssl = slice(g * CAP_S, (g + 1) * CAP_S)
        xgS = np.ascontiguousarray(xTq[:, ssl])
        scw = np.ones((P, CAP_S // P), f32)
        meta.append((toks[eA], counts[eA], np.asarray(cws[eA], f32), eA,
                     toks[eB], counts[eB], np.asarray(cws[eB], f32), eB))
        hvA = halves(w1[eA], b1[eA], w3[eA], b3[eA], w2[eA], b2[eA])
        hvB = halves(w1[eB], b1[eB], w3[eB], b3[eB], w2[eB], b2[eB])
        for h in range(2):
            c = 2 * g + h
            m = in_maps[c]
            assemble(m, "a_", hvA[h], xgA, cwA)
            assemble(m, "b_", hvB[h], eo_split(xgB, CAP_B), cwB)
            assemble(m, "s_", sh_halves[h], eo_split(xgS, CAP_S), scw)
    return in_maps, meta, None


_PROGS = {}


def _get_program(kind):
    if kind not in _PROGS:
        _PROGS[kind] = {"sparse": _build_sparse, "dense": _build_dense}[kind]()
    return _PROGS[kind]


def kernel(x, gate_w, gate_b, w1, b1, w3, b3, w2, b2,
           sw1, sb1, sw3, sb3, sw2, sb2, _trace=False, _results=None,
           _force_dense=False):
    kw = {}
    if _trace:
        kw = dict(trace=True, trace_cores=list(range(NCORES)))
    args = (x, gate_w, gate_b, w1, b1, w3, b3, w2, b2,
            sw1, sb1, sw3, sb3, sw2, sb2)
    if not _force_dense:
        in_maps, meta, _ = _prep_sparse(*args)
    else:
        in_maps = None
    if in_maps is not None:
        nc = _get_program("sparse")
        res = run_bass_kernel_spmd(
            nc, in_maps, core_ids=list(range(NCORES)), **kw)
        if _results is not None:
            _results.append(res)
        f32 = np.float32
        out = np.zeros((T, D), f32)
        for g in range(NGROUPS):
            r0, r1 = res.results[2 * g], res.results[2 * g + 1]
            out[g * CAP_S:(g + 1) * CAP_S] = (
                r0["s_out"].astype(f32) + r1["s_out"].astype(f32))
        out += np.asarray(sb2, f32)          # shared-expert down bias
        for g in range(NGROUPS):
            r0, r1 = res.results[2 * g], res.results[2 * g + 1]
            tA, nA, cwA, eA, tB, nB, cwB, eB = meta[g]
            # a_out is the transposed raw yT [D, CAP_A]; apply cw here
            yTA = (r0["a_out"].astype(f32) + r1["a_out"].astype(f32))
            out[tA] += cwA[:nA, None] * (
                yTA.T[:nA] + np.asarray(b2[eA], f32)[None, :])
            out[tB] += (r0["b_out"][:nB].astype(f32)
                        + r1["b_out"][:nB].astype(f32)
                        + cwB[:nB, None] * np.asarray(b2[eB], f32))
        return out.reshape(np.asarray(x).shape).astype(np.float32)

    # dense fallback
    in_maps = _prep_dense(*args)
    nc = _get_program("dense")
    res = run_bass_kernel_spmd(nc, in_maps, core_ids=list(range(NCORES)), **kw)
    if _results is not None:
        _results.append(res)
    acc = np.zeros((T, D), np.float32)
    for c in range(NCORES):
        acc += res.results[c]["out"]
    return acc.reshape(np.asarray(x).shape).astype(np.float32)


# ---------------------------------------------------------------------------
# dense all-on-device fallback (V1): every core runs its expert over all
# tokens, masked by on-device gate weights; shared expert sharded on 2F.
# ---------------------------------------------------------------------------

TCH = 512
NTH = 2
TH = T // NTH
DKF = D // P
FIF = F // P


def _build_dense():
    nc = bacc.Bacc(
        "TRN2", target_bir_lowering=False, debug=False, num_devices=NCORES
    )
    aps = {}

    def inp(name, shape, dt=F32):
        aps[name] = nc.dram_tensor(name, shape, dt, kind="ExternalInput").ap()

    inp("xT", [D, T], F32R)
    inp("gw", [P, DKF * E], F32R)
    inp("gb", [P, E])
    inp("sel", [P, E])
    for n in ("w1g", "w1l", "w3g", "w3l"):
        inp(n, [FIF, P, DKF, P], F32R)
    for n in ("b1g", "b1l", "b3g", "b3l"):
        inp(n, [P, FIF + 1])
    inp("w2T", [F, D], F32R)
    inp("b2r", [1, D], F32R)
    for n in ("sw1g", "sw1l", "sw3g", "sw3l"):
        inp(n, [P, DKF, P], F32R)
    inp("sw2T", [P, D], F32R)
    inp("sb2r", [1, D], F32R)
    inp("ones", [1, P], F32R)
    aps["out"] = nc.dram_tensor("out", [T, D], F32, kind="ExternalOutput").ap()

    with tile.TileContext(nc) as tc:
        _emit_dense(tc, aps)
    nc.compile()
    return nc


def _emit_dense(tc, aps):
    nc = tc.nc
    ctx = ExitStack()

    with ctx:
        const = ctx.enter_context(tc.tile_pool(name="const", bufs=1))

        xsb = []
        for dk in range(DKF):
            t = const.tile([P, T], F32R, tag=f"x{dk}")
            nc.sync.dma_start(t[:], aps["xT"][dk * P:(dk + 1) * P, :])
            xsb.append(t)

        def load_const(name, shape, dt=F32):
            t = const.tile(shape, dt, tag=name)
            nc.sync.dma_start(t[:], aps[name][:])
            return t

        gw_sb = load_const("gw", [P, DKF * E], F32R)
        gb_sb = load_const("gb", [P, E])
        sel_sb = load_const("sel", [P, E])
        bcols = {n: load_const(n, [P, FIF + 1])
                 for n in ("b1g", "b1l", "b3g", "b3l")}
        b2r_sb = load_const("b2r", [1, D], F32R)
        sb2r_sb = load_const("sb2r", [1, D], F32R)
        sw2T_sb = load_const("sw2T", [P, D], F32R)
        ssw = {}
        for name in ("sw1g", "sw1l", "sw3g", "sw3l"):
            t = const.tile([P, DKF, P], F32R, tag=name)
            nc.sync.dma_start(t[:], aps[name][:])
            ssw[name] = t

        ones = const.tile([1, P], F32R, tag="ones")
        nc.sync.dma_start(ones[:], aps["ones"][:])
        ident = const.tile([E, E], F32, tag="ident")
        nc.vector.memset(ident[:], 0.0)
        from concourse.masks import make_identity
        make_identity(nc, ident[:], nomemset=True)

        cw = const.tile([P, T // P], F32, tag="cw")

        # ---- gate ----
        with tc.tile_pool(name="psG", bufs=2, space="PSUM") as psG, \
             tc.tile_pool(name="gtmp", bufs=1) as gtmp:
            NC = T // P
            logits_tb = const.tile([P, NC * E], F32, tag="logits_tb")
            logitsT = gtmp.tile([E, T], F32, tag="logitsT")
            for tch in range(T // TCH):
                pg = psG.tile([E, TCH], F32, tag="pslog")
                for dk in range(DKF):
                    nc.tensor.matmul(
                        pg[:],
                        (gw_sb[:, dk * E:(dk + 1) * E]),
                        (xsb[dk][:, tch * TCH:(tch + 1) * TCH]),
                        start=(dk == 0), stop=(dk == DKF - 1),
                    )
                nc.scalar.copy(logitsT[:, tch * TCH:(tch + 1) * TCH], pg[:])
            for j in range(NC):
                pt = psG.tile([P, E], F32, tag="pstr")
                nc.tensor.transpose(
                    pt[:], logitsT[:, j * P:(j + 1) * P], ident[:])
                nc.scalar.copy(logits_tb[:, j * E:(j + 1) * E], pt[:])

            eL = gtmp.tile([P, NC * E], F32, tag="eL")
            nc.scalar.activation(eL[:], logits_tb[:], AF.Exp)
            e3 = eL[:].rearrange("p (c e) -> p c e", e=E)
            ssum = gtmp.tile([P, NC], F32, tag="ssum")
            nc.vector.reduce_sum(ssum[:], e3, axis=mybir.AxisListType.X)
            rs = gtmp.tile([P, NC], F32, tag="rs")
            nc.vector.reciprocal(rs[:], ssum[:])
            scores = gtmp.tile([P, NC * E], F32, tag="scores")
            s3 = scores[:].rearrange("p (c e) -> p c e", e=E)
            nc.vector.tensor_mul(
                s3, e3, rs[:, :, None].broadcast_to((P, NC, E)))
            biased = gtmp.tile([P, NC * E], F32, tag="biased")
            bi3 = biased[:].rearrange("p (c e) -> p c e", e=E)
            nc.vector.tensor_add(
                bi3, s3, gb_sb[:, None, :].broadcast_to((P, NC, E)))
            m1 = gtmp.tile([P, NC], F32, tag="m1")
            nc.vector.reduce_max(m1[:], bi3, axis=mybir.AxisListType.X)
            mask1 = gtmp.tile([P, NC * E], F32, tag="mask1")
            mk3 = mask1[:].rearrange("p (c e) -> p c e", e=E)
            nc.vector.tensor_tensor(
                mk3, bi3, m1[:, :, None].broadcast_to((P, NC, E)), OP.is_ge)
            biased2 = gtmp.tile([P, NC * E], F32, tag="biased2")
            b23 = biased2[:].rearrange("p (c e) -> p c e", e=E)
            nc.vector.scalar_tensor_tensor(
                out=b23, in0=mk3, scalar=-1e30, in1=bi3,
                op0=OP.mult, op1=OP.add)
            m2 = gtmp.tile([P, NC], F32, tag="m2")
            nc.vector.reduce_max(m2[:], b23, axis=mybir.AxisListType.X)
            mask2 = gtmp.tile([P, NC * E], F32, tag="mask2")
            mq3 = mask2[:].rearrange("p (c e) -> p c e", e=E)
            nc.vector.tensor_tensor(
                mq3, bi3, m2[:, :, None].broadcast_to((P, NC, E)), OP.is_ge)
            cwf = gtmp.tile([P, NC * E], F32, tag="cwf")
            cf3 = cwf[:].rearrange("p (c e) -> p c e", e=E)
            nc.vector.tensor_mul(cf3, s3, mq3)
            nc.vector.tensor_mul(
                cf3, cf3, sel_sb[:, None, :].broadcast_to((P, NC, E)))
            nc.vector.reduce_sum(cw[:], cf3, axis=mybir.AxisListType.X)

        # ---- main ----
        wA = ctx.enter_context(tc.tile_pool(name="wA", bufs=3))
        w2p = ctx.enter_context(tc.tile_pool(name="w2p", bufs=3))
        apool = ctx.enter_context(tc.tile_pool(name="apool", bufs=1))
        hpool = ctx.enter_context(tc.tile_pool(name="hpool", bufs=2))
        outp = ctx.enter_context(tc.tile_pool(name="outp", bufs=3))
        psA = ctx.enter_context(tc.tile_pool(name="psA", bufs=2, space="PSUM"))
        psB = ctx.enter_context(tc.tile_pool(name="psB", bufs=2, space="PSUM"))
        psS = ctx.enter_context(tc.tile_pool(name="psS", bufs=2, space="PSUM"))

        afc = FIF + 1
        for th in range(NTH):
            tbase = th * TH
            atiles = []
            for fi in range(afc):
                at = apool.tile([P, TH], F32R, tag=f"a{fi}")
                atiles.append(at)
                if fi < FIF:
                    wt = {}
                    for nm in ("w1g", "w1l", "w3g", "w3l"):
                        t = wA.tile([P, DKF, P], F32R, tag=nm)
                        nc.sync.dma_start(t[:], aps[nm][fi])
                        wt[nm] = t
                    w_g1, w_l1 = wt["w1g"], wt["w1l"]
                    w_g3, w_l3 = wt["w3g"], wt["w3l"]
                else:
                    w_g1, w_l1 = ssw["sw1g"], ssw["sw1l"]
                    w_g3, w_l3 = ssw["sw3g"], ssw["sw3l"]
                bc1g = bcols["b1g"][:, fi:fi + 1]
                bc1l = bcols["b1l"][:, fi:fi + 1]
                bc3g = bcols["b3g"][:, fi:fi + 1]
                bc3l = bcols["b3l"][:, fi:fi + 1]

                for tt in range(TH // TCH):
                    tsl = slice(tt * TCH, (tt + 1) * TCH)
                    gsl = slice(tbase + tt * TCH, tbase + (tt + 1) * TCH)

                    def hpsum(wtile, ptag):
                        ps = psA.tile([P, TCH], F32, tag=ptag)
                        for dk in range(DKF):
                            nc.tensor.matmul(
                                ps[:], (wtile[:, dk, :]),
                                (xsb[dk][:, gsl]),
                                start=(dk == 0), stop=(dk == DKF - 1))
                        return ps

                    pg1 = hpsum(w_g1, "pA")
                    t1 = hpool.tile([P, TCH], F32, tag="tcp")
                    nc.scalar.activation(t1[:], pg1[:], AF.Identity, bias=bc1g)
                    pg3 = hpsum(w_g3, "pB")
                    hg = hpool.tile([P, TCH], F32, tag="hh")
                    nc.vector.scalar_tensor_tensor(
                        out=hg[:], in0=pg3[:], scalar=bc3g, in1=t1[:],
                        op0=OP.add, op1=OP.mult)
                    nc.vector.tensor_scalar_min(hg[:], hg[:], LIMIT)
                    gs = hpool.tile([P, TCH], F32, tag="gs")
                    nc.scalar.activation(gs[:], hg[:], AF.Silu, scale=ALPHA)

                    pl1 = hpsum(w_l1, "pA")
                    t2 = hpool.tile([P, TCH], F32, tag="tcp")
                    nc.scalar.activation(t2[:], pl1[:], AF.Identity, bias=bc1l)
                    pl3 = hpsum(w_l3, "pB")
                    hl = hpool.tile([P, TCH], F32, tag="hh")
                    nc.vector.scalar_tensor_tensor(
                        out=hl[:], in0=pl3[:], scalar=bc3l, in1=t2[:],
                        op0=OP.add, op1=OP.mult)
                    nc.vector.tensor_scalar(
                        out=hl[:], in0=hl[:], scalar1=LIMIT, scalar2=-LIMIT,
                        op0=OP.min, op1=OP.max)
                    nc.vector.tensor_scalar(
                        out=hl[:], in0=hl[:], scalar1=1.0 / ALPHA,
                        scalar2=1.0 / ALPHA, op0=OP.mult, op1=OP.add)
                    nc.vector.tensor_mul(atiles[fi][:, tsl], gs[:], hl[:])

            for tp in range(TH // P):
                j = th * (TH // P) + tp
                tsl = slice(tp * P, (tp + 1) * P)
                for dch in range(D // TCH):
                    dsl = slice(dch * TCH, (dch + 1) * TCH)
                    pB = psB.tile([P, TCH], F32, tag="pB2")
                    nc.tensor.matmul(pB[:], (ones[:]),
                                     (b2r_sb[0:1, dsl]),
                                     start=True, stop=False)
                    for fi in range(FIF):
                        wt2 = w2p.tile([P, TCH], F32R, tag="w2t")
                        nc.sync.dma_start(
                            wt2[:], aps["w2T"][fi * P:(fi + 1) * P, dsl])
                        nc.tensor.matmul(
                            pB[:], (atiles[fi][:, tsl]), (wt2[:]),
                            start=False, stop=(fi == FIF - 1))
                    pS = psS.tile([P, TCH], F32, tag="pS")
                    nc.tensor.matmul(pS[:], (ones[:]),
                                     (sb2r_sb[0:1, dsl]),
                                     start=True, stop=False)
                    nc.tensor.matmul(
                        pS[:], (atiles[FIF][:, tsl]), (sw2T_sb[:, dsl]),
                        start=False, stop=True)
                    ot = outp.tile([P, TCH], F32, tag="ot")
                    nc.vector.tensor_scalar_mul(ot[:], pB[:], cw[:, j:j + 1])
                    nc.vector.tensor_add(ot[:], pS[:], ot[:])
                    nc.sync.dma_start(
                        aps["out"][tbase + tp * P:tbase + (tp + 1) * P, dsl],
                        ot[:])


def _prep_dense(x, gate_w, gate_b, w1, b1, w3, b3, w2, b2,
                sw1, sb1, sw3, sb3, sw2, sb2):
    f32 = np.float32
    xt = np.asarray(x, f32).reshape(T, D)
    xT = np.ascontiguousarray(xt.T)
    gwT = np.asarray(gate_w, f32).T
    gw_sb = np.ascontiguousarray(
        gwT.reshape(DKF, P, E).transpose(1, 0, 2).reshape(P, DKF * E))
    gb_bc = np.ascontiguousarray(
        np.broadcast_to(np.asarray(gate_b, f32), (P, E)))

    sw1 = np.asarray(sw1, f32)
    sw3 = np.asarray(sw3, f32)
    sw2T = np.asarray(sw2, f32).T
    sb1 = np.asarray(sb1, f32)
    sb3 = np.asarray(sb3, f32)
    sb2 = np.asarray(sb2, f32)

    def fwarr(w):      # [F, D] -> [FIF, P, DKF, P]
        return np.ascontiguousarray(
            w.T.reshape(DKF, P, FIF, P).transpose(2, 1, 0, 3))

    def swarr(w_sl):
        return np.ascontiguousarray(
            w_sl.T.reshape(DKF, P, P).transpose(1, 0, 2))

    def bcol2(b, sb_sl):
        return np.ascontiguousarray(
            np.concatenate([b.reshape(FIF, P).T, sb_sl[:, None]], axis=1))

    in_maps = []
    for c in range(NCORES):
        sel = np.zeros((P, E), f32)
        sel[:, c] = 1.0
        w1c = np.asarray(w1[c], f32)
        w3c = np.asarray(w3[c], f32)
        b1c = np.asarray(b1[c], f32)
        b3c = np.asarray(b3[c], f32)
        fsl = slice(c * P, (c + 1) * P)
        m = {
            "xT": xT, "gw": gw_sb, "gb": gb_bc, "sel": sel,
            "w1g": fwarr(w1c[0::2]), "w1l": fwarr(w1c[1::2]),
            "w3g": fwarr(w3c[0::2]), "w3l": fwarr(w3c[1::2]),
            "b1g": bcol2(b1c[0::2], sb1[0::2][fsl]),
            "b1l": bcol2(b1c[1::2], sb1[1::2][fsl]),
            "b3g": bcol2(b3c[0::2], sb3[0::2][fsl]),
            "b3l": bcol2(b3c[1::2], sb3[1::2][fsl]),
            "w2T": np.ascontiguousarray(np.asarray(w2[c], f32).T),
            "b2r": np.asarray(b2[c], f32)[None, :],
            "sw1g": swarr(sw1[0::2][fsl]), "sw1l": swarr(sw1[1::2][fsl]),
            "sw3g": swarr(sw3[0::2][fsl]), "sw3l": swarr(sw3[1::2][fsl]),
            "sw2T": np.ascontiguousarray(sw2T[fsl]),
            "sb2r": (sb2 if c == 0 else np.zeros_like(sb2))[None, :],
            "ones": np.ones((1, P), f32),
        }
        in_maps.append(m)
    return in_maps


if __name__ == "__main__":
    rng = np.random.RandomState(0)
    sd = 1 / 32.0
    ins = {
        "x": rng.randn(2, 1024, 1024).astype(np.float32),
        "gate_w": (rng.randn(E, D) * sd).astype(np.float32),
        "gate_b": (rng.randn(E) * 0.01).astype(np.float32),
        "w1": (rng.randn(E, 2 * F, D) * sd).astype(np.float32),
        "b1": (rng.randn(E, 2 * F) * 0.01).astype(np.float32),
        "w3": (rng.randn(E, 2 * F, D) * sd).astype(np.float32),
        "b3": (rng.randn(E, 2 * F) * 0.01).astype(np.float32),
        "w2": (rng.randn(E, D, F) * sd).astype(np.float32),
        "b2": (rng.randn(E, D) * 0.01).astype(np.float32),
        "sw1": (rng.randn(2 * F, D) * sd).astype(np.float32),
        "sb1": (rng.randn(2 * F) * 0.01).astype(np.float32),
        "sw3": (rng.randn(2 * F, D) * sd).astype(np.float32),
        "sb3": (rng.randn(2 * F) * 0.01).astype(np.float32),
        "sw2": (rng.randn(D, F) * sd).astype(np.float32),
        "sb2": (rng.randn(D) * 0.01).astype(np.float32),
    }
    out = kernel(**ins)
    print("OK", out.shape, out.dtype, np.abs(out).mean())



# revision 59
# speedup vs baseline: 1.0170x; 1.0170x over previous
"""MoE routing kernel for Trainium2 (8 NeuronCores, paired-expert F-sharding).

Sharding strategy:
  - The host computes the gate (same math as the reference, on CPU jax so
    tie-breaking matches bit-for-bit) and pairs experts heavy-with-light
    by routed load (e.g. 592+453). Core pair (2g, 2g+1) owns expert pair
    g: each core of the pair processes ALL of both experts' routed tokens
    but only HALF of the F channels (F-sharding), so per-core compute is
    uniform across the chip regardless of routing imbalance.
  - The shared expert is token-sharded across the 4 groups (512 tokens
    each) and F-sharded across the pair, same emitter.
  - Every core therefore runs 3 FFN batches: expert-A tokens (cap 656),
    expert-B tokens (cap 528), shared slice (512) — 1696 columns of
    half-F work ≈ the ideal 64.4 GFLOP / 8 cores.
  - All matmul operands are bf16 (fp32 PSUM accumulate): halves HBM
    traffic; end-to-end quantization error ~5e-3 absmax-rel.
  - Unshard on host: partial outputs of the two F-halves (bf16) add;
    shared slices concatenate; routed outputs scatter-add by token index.
    Combine weights are applied on-device for b_/s_ (ACT/DVE scale); the
    a_ batch stores transposed raw yT[D, cap] (stage-B computes cap=648
    token columns instead of 768 padded rows) and the host applies cw
    plus the down-proj biases during combination.

Schedule/overlap techniques (all measured on HW):
  - fp32 warmup matmuls on memset data release the PE's HAM clock gate
    (1.2 -> 2.4 GHz) before the first real matmul and bridge the
    opening-DMA latency window; bf16 warmups do NOT trip the HAM.
  - The opening loads only chunk0's x columns per-dk in consumption
    order; the fi0 loop is dk-outer into 4 parallel PSUM banks so each
    arriving x tile immediately feeds 4 matmuls (just-in-time start).
  - The 16 DMA rings fair-share among outstanding transfers, so all
    mid-kernel loads are paced: issued on the ACT queue at emission
    points behind compute ops, keeping <=2 transfers in flight and
    arrival ~= need order. Result: a gap-free warm matmul stream within
    ~1.5% of the bf16 1-column/cycle architectural floor.

A dense all-on-device fallback (every core processes all tokens through
its expert, masked by gate weights computed on-device) is kept for the
(never observed) case that a pair's load exceeds capacity.
"""

import numpy as np
from contextlib import ExitStack

import ml_dtypes

import concourse.bass as bass
import concourse.mybir as mybir
import concourse.tile as tile
from concourse import bacc
from concourse.bass_utils import run_bass_kernel_spmd

# Problem dims (hardcoded per contract)
E = 8
D = 1024
F = 1024
T = 2048          # B*S = 2*1024
P = 128
DK = D // P       # 8 k-chunks over D
FH = F // 2       # 512 channels per core (F-shard half)
FI = FH // P      # 4 f-chunks per branch per core
ALPHA = 1.702
LIMIT = 7.0
NCORES = 8
NGROUPS = 4
CAP_A = 648       # cap for the heavier expert of each pair (max load 642)
CAP_B = 512       # cap for the lighter expert (max load 508; 4 full tiles)
CAP_S = T // NGROUPS  # shared-expert tokens per group

F32 = mybir.dt.float32
F32R = mybir.dt.float32r
BF16 = mybir.dt.bfloat16
AF = mybir.ActivationFunctionType
OP = mybir.AluOpType

BF = ml_dtypes.bfloat16


FAST_TEARDOWN = True


class _LeanTC(tile.TileContext):
    """TileContext with a lighter end-of-program teardown.

    The stock teardown (drain + barrier + gpsimd dma_reset + sem_clear +
    barrier) measures ~6us of pure serialized tail on the slow gpsimd
    sequencer AFTER the last data movement. The dma_reset (a Pool-engine
    InstDrain over the kernel's semaphore range) is only needed so a
    subsequent execution of the same loaded NEFF sees quiesced DMA
    state; we run each program once per process, so skip it and keep
    only the semaphore clear."""

    def _drain_and_barrier(self, tick_clock, wait_clock):
        from concourse.vector_clock import ScopedClock
        from concourse.bass import compact_to_ranges
        nc = self.nc
        drain_inst = nc.sync.drain()
        wait_clock.add_sem_waits(
            drain_inst.ins, ScopedClock({None: tick_clock.global_clock}))
        nc.all_engine_barrier()
        popped = nc._tile_sem_poison_stack.pop()
        assert popped is self._sem_poison
        sems = list(self.sems.allocated().values())
        sem_nums = [s.num if hasattr(s, "num") else int(s) for s in sems]
        for rng in compact_to_ranges(sem_nums):
            assert nc._state.free_isdisjoint(rng)
            nc.gpsimd.sem_clear(rng)
        nc._state.prepend_free_semaphores(sem_nums)
        for poison_set in nc._tile_sem_poison_stack:
            poison_set.update(sem_nums)
        nc.all_engine_barrier()


def _chunks(n):
    # near-equal chunks <= 512 (a tiny tail chunk wastes the ~60-cycle
    # matmul issue floor, so balance instead: 528 -> 264+264)
    k = -(-n // 512)
    base = n // k
    rem = n - base * k
    out = []
    o = 0
    for i in range(k):
        s = base + (1 if i < rem else 0)
        out.append((o, s))
        o += s
    return out


# ---------------------------------------------------------------------------
# generic FFN-batch emitter (half-F):
#   out[cap, D] = cw * (swiglu((xT@w1+b1)*(xT@w3+b3)) @ (w2T/alpha) + b2)
# where swiglu' returns alpha*a (the 1/alpha is folded into w2T on host).
# ---------------------------------------------------------------------------

def _emit_warmup(tc, pools, n_mms=13):
    """Dummy matmuls on memset data, issued before any input-dependent
    work. They run during the ~5us opening-DMA latency window, so the
    PE's HAM clock gate (4096-cycle activity window, ~3.4us) releases
    to 8/8 before the first real matmul — otherwise the opening ~3.4us
    of real matmuls run at 1.2 GHz instead of 2.4. Counterintuitively
    the tiles MUST be fp32: measured on HW, a stream of 28 bf16 N=128
    matmuls (~50% PE-array duty due to interleaved LDWEIGHTS) never
    tripped the HAM activity window, while fp32 (4 cycles/row, ~80%
    duty) trips it ~2.8us after the first warmup MM. 8 fp32 MMs =
    ~3.4us, ending about when the first real weight/x tiles land."""
    nc = tc.nc
    wA, w2p, apool, hpool, outp, psA, psB = pools
    wz = hpool.tile([P, P], F32, tag="wz")
    xz = hpool.tile([P, P], F32, tag="xz")
    nc.gpsimd.memset(wz[:], 0.0)
    nc.gpsimd.memset(xz[:], 0.0)
    for _ in range(n_mms):
        ps = psA.tile([P, 512], F32, tag="pA")
        nc.tensor.matmul(ps[:, :P], wz[:], xz[:], start=True, stop=True)


WSETS = ("w1g", "w1l", "w3g", "w3l")


def _mk_loader(tc, pools, aps):
    """Tile-allocating DMA closures + per-batch state.

    Key scheduling fact (measured): the 16 DMA rings FAIR-SHARE among
    all outstanding transfers, so a prefetch issued early delays the
    first-needed transfer proportionally. Mid-kernel loads are therefore
    issued on the ACT (scalar) queue at chosen emission points between
    compute ops — the in-order sequencer only reaches the dma_start
    after the preceding compute retires, pacing issues to ~need order
    with at most ~2 transfers in flight."""
    nc = tc.nc
    wA, w2p, apool, hpool, outp, psA, psB = pools
    state = {}

    def st(pref):
        return state.setdefault(pref, {"wcs": {}})

    def wcat(pref, fi, eng=None):
        wc = wA.tile([P, 4, DK, P], BF16, tag="wcat")
        (eng or nc.scalar).dma_start(wc[:], aps[pref + "wcat"][fi])
        st(pref)["wcs"][fi] = {nm: wc[:, j] for j, nm in enumerate(WSETS)}

    def xe(pref, cap, eng=None):
        t = apool.tile([P, DK // 2, cap], BF16, tag=pref + "xe")
        (eng or nc.scalar).dma_start(t[:], aps[pref + "xe"][:])
        st(pref)["xe"] = t

    def xo(pref, cap, eng=None):
        t = apool.tile([P, DK // 2, cap], BF16, tag=pref + "xo")
        (eng or nc.scalar).dma_start(t[:], aps[pref + "xo"][:])
        s = st(pref)
        s["xo"] = t
        xet = s["xe"]

        def xat(dk, to, ts):
            return (xet if dk % 2 == 0 else t)[:, dk // 2, to:to + ts]

        s["xat"] = xat

    def ballcw(pref, cap, eng=None):
        ntt = (cap + P - 1) // P
        t = apool.tile([P, 4 * FI + ntt], F32, tag=pref + "ballcw")
        (eng or nc.scalar).dma_start(t[:], aps[pref + "ballcw"][:])
        st(pref)["ballcw"] = t

    def w2T(pref, eng=None):
        # a_ uses the transposed stage-B layout (same byte size)
        shape = [P, FI, DK, P] if pref == "a_" else [P, FI, D]
        t = w2p.tile(shape, BF16, tag="w2t")
        (eng or nc.scalar).dma_start(t[:], aps[pref + "w2T"][:])
        st(pref)["w2t"] = t

    return state, dict(wcat=wcat, xe=xe, xo=xo, ballcw=ballcw, w2T=w2T)


def _emit_opening(tc, pools, aps, state, load, cap):
    """Opening flood for the first batch: per-dk chunk0-only x tiles +
    per-wset fi0 weight tiles interleaved across both HWDGE queues in
    CONSUMPTION order — the fi0-chunk0 loop is dk-outer, so each
    arriving x tile immediately enables 4 matmuls and the PE streams
    just-in-time behind the DMAs. Only chunk0's x columns ride the
    critical window (~1.7MB instead of 2.3MB); chunk1's columns follow
    as two merged transfers and land during chunk0 compute. The fi1
    weights are NOT issued here — the rings fair-share among all
    outstanding transfers, so any early prefetch delays the whole
    opening; fi1 is paced after chunk0's chain instead. The gpsimd
    queue is never used: it is a software-DGE path that trickles data
    and (measured) starves everything sharing the rings."""
    nc = tc.nc
    wA, w2p, apool, hpool, outp, psA, psB = pools
    s = state.setdefault("a_", {"wcs": {}})
    (c0o, c0n), (c1o, c1n) = _chunks(cap)

    weng = {"w1g": nc.sync, "w1l": nc.sync,
            "w3g": nc.scalar, "w3l": nc.scalar}
    wt0 = {}
    for nm in ("w1g", "w3g", "w1l", "w3l"):
        t = wA.tile([P, DK, P], BF16, tag="w0" + nm)
        weng[nm].dma_start(t[:], aps["a_" + nm][:])
        wt0[nm] = t[:]
    s["wcs"][0] = wt0

    xc0 = []
    for dk in range(DK):
        t = apool.tile([P, c0n], BF16, tag=f"a_x{dk}")
        eng = nc.sync if dk % 2 == 0 else nc.scalar
        eng.dma_start(
            t[:], aps["a_xT"][dk * P:(dk + 1) * P, c0o:c0o + c0n])
        xc0.append(t)
    xc1e = apool.tile([P, DK // 2, c1n], BF16, tag="a_xc1e")
    nc.sync.dma_start(xc1e[:], aps["a_xc1e"][:])
    xc1o = apool.tile([P, DK // 2, c1n], BF16, tag="a_xc1o")
    nc.scalar.dma_start(xc1o[:], aps["a_xc1o"][:])

    def xat(dk, to, ts):
        if to == c0o:
            return xc0[dk][:, :ts]
        return (xc1e if dk % 2 == 0 else xc1o)[:, dk // 2, :ts]

    s["xat"] = xat
    load["ballcw"]("a_", cap, nc.scalar)


def _emit_stage_a(tc, pools, pref, aps, cap, tiles, first=False,
                  pace=None):
    """pace: {position: [thunk]} — thunks (paced ACT-queue dma_starts)
    run after the chunk at that position in the (fi, chunk) chain."""
    nc = tc.nc
    wA, w2p, apool, hpool, outp, psA, psB = pools
    wcs, xat = tiles["wcs"], tiles["xat"]
    ball = tiles["ballcw"]
    pace = pace or {}
    ntt = (cap + P - 1) // P

    def swiglu_chain(pg1, pg3, pl1, pl3, ts, biases, out_ap):
        bc1g, bc1l, bc3g, bc3l = biases
        t1 = hpool.tile([P, 512], F32, tag="tcp")
        nc.scalar.activation(t1[:, :ts], pg1[:, :ts], AF.Identity,
                             bias=bc1g)
        hg = hpool.tile([P, 512], F32, tag="hh")
        nc.vector.scalar_tensor_tensor(
            out=hg[:, :ts], in0=pg3[:, :ts], scalar=bc3g, in1=t1[:, :ts],
            op0=OP.add, op1=OP.mult)
        nc.vector.tensor_scalar_min(hg[:, :ts], hg[:, :ts], LIMIT)
        gs = hpool.tile([P, 512], F32, tag="gs")
        nc.scalar.activation(gs[:, :ts], hg[:, :ts], AF.Silu, scale=ALPHA)
        t2 = hpool.tile([P, 512], F32, tag="tcp")
        nc.scalar.activation(t2[:, :ts], pl1[:, :ts], AF.Identity,
                             bias=bc1l)
        hl = hpool.tile([P, 512], F32, tag="hh")
        nc.vector.scalar_tensor_tensor(
            out=hl[:, :ts], in0=pl3[:, :ts], scalar=bc3l, in1=t2[:, :ts],
            op0=OP.add, op1=OP.mult)
        nc.vector.tensor_scalar(
            out=hl[:, :ts], in0=hl[:, :ts], scalar1=LIMIT, scalar2=-LIMIT,
            op0=OP.min, op1=OP.max)
        # a = (hl + 1) * gs   (the 1/alpha lives in w2T)
        nc.vector.scalar_tensor_tensor(
            out=out_ap, in0=hl[:, :ts], scalar=1.0,
            in1=gs[:, :ts], op0=OP.add, op1=OP.mult)

    pos = 0

    def run_pace():
        for thunk in pace.get(pos, ()):
            thunk()

    atiles = []
    for fi in range(FI):
        at = apool.tile([P, cap], BF16, tag=f"{pref}a{fi}")
        atiles.append(at)
        wt = wcs[fi]
        biases = (ball[:, 0 * FI + fi:0 * FI + fi + 1],
                  ball[:, 1 * FI + fi:1 * FI + fi + 1],
                  ball[:, 2 * FI + fi:2 * FI + fi + 1],
                  ball[:, 3 * FI + fi:3 * FI + fi + 1])

        chunks = _chunks(cap)
        if fi == 0 and first:
            # JIT opening: dk-outer over 4 parallel PSUM accumulations so
            # each x tile feeds the PE the moment its DMA lands.
            (to, ts) = chunks[0]
            tsl = slice(to, to + ts)
            pg1 = psA.tile([P, 512], F32, tag="pA")
            pg3 = psB.tile([P, 512], F32, tag="pB2")
            pl1 = psA.tile([P, 512], F32, tag="pB")
            pl3 = psB.tile([P, 512], F32, tag="pB2")
            pmap = {"w1g": pg1, "w3g": pg3, "w1l": pl1, "w3l": pl3}
            for dk in range(DK):
                for nm in ("w1g", "w3g", "w1l", "w3l"):
                    nc.tensor.matmul(
                        pmap[nm][:, :ts], (wt[nm][:, dk, :]),
                        (xat(dk, to, ts)),
                        start=(dk == 0), stop=(dk == DK - 1))
            swiglu_chain(pg1, pg3, pl1, pl3, ts, biases, at[:, tsl])
            run_pace()
            pos += 1
            chunks = chunks[1:]

        for (to, ts) in chunks:
            tsl = slice(to, to + ts)

            def hpsum(wtile, ptag):
                ps = psA.tile([P, 512], F32, tag=ptag)
                for dk in range(DK):
                    nc.tensor.matmul(
                        ps[:, :ts], (wtile[:, dk, :]),
                        (xat(dk, to, ts)),
                        start=(dk == 0), stop=(dk == DK - 1))
                return ps

            pg1 = hpsum(wt["w1g"], "pA")
            t1 = hpool.tile([P, 512], F32, tag="tcp")
            nc.scalar.activation(t1[:, :ts], pg1[:, :ts], AF.Identity,
                                 bias=biases[0])
            pg3 = hpsum(wt["w3g"], "pB")
            hg = hpool.tile([P, 512], F32, tag="hh")
            nc.vector.scalar_tensor_tensor(
                out=hg[:, :ts], in0=pg3[:, :ts], scalar=biases[2],
                in1=t1[:, :ts], op0=OP.add, op1=OP.mult)
            nc.vector.tensor_scalar_min(hg[:, :ts], hg[:, :ts], LIMIT)
            gs = hpool.tile([P, 512], F32, tag="gs")
            nc.scalar.activation(gs[:, :ts], hg[:, :ts], AF.Silu, scale=ALPHA)

            pl1 = hpsum(wt["w1l"], "pA")
            t2 = hpool.tile([P, 512], F32, tag="tcp")
            nc.scalar.activation(t2[:, :ts], pl1[:, :ts], AF.Identity,
                                 bias=biases[1])
            pl3 = hpsum(wt["w3l"], "pB")
            hl = hpool.tile([P, 512], F32, tag="hh")
            nc.vector.scalar_tensor_tensor(
                out=hl[:, :ts], in0=pl3[:, :ts], scalar=biases[3],
                in1=t2[:, :ts], op0=OP.add, op1=OP.mult)
            nc.vector.tensor_scalar(
                out=hl[:, :ts], in0=hl[:, :ts], scalar1=LIMIT, scalar2=-LIMIT,
                op0=OP.min, op1=OP.max)
            # a = (hl + 1) * gs   (the 1/alpha lives in w2T)
            nc.vector.scalar_tensor_tensor(
                out=atiles[fi][:, tsl], in0=hl[:, :ts], scalar=1.0,
                in1=gs[:, :ts], op0=OP.add, op1=OP.mult)
            run_pace()
            pos += 1
    tiles["atiles"] = atiles


def _emit_stage_bT(tc, pools, pref, aps, cap, tiles):
    """Transposed stage B for the capacity-padded a_ batch: computes
    yT[D, cap] = (w2T)ᵀ-blocks @ a instead of aᵀ @ w2T. PE cost scales
    with the token-column count (cap=648) rather than the padded
    ceil(cap/128)*128 = 768, saving ~1.5us. The combine weight cannot
    be applied per-COLUMN on device, so for this batch raw yT partials
    are stored and the host applies cw during the scatter-add."""
    nc = tc.nc
    wA, w2p, apool, hpool, outp, psA, psB = pools
    atiles, w2d = tiles["atiles"], tiles["w2t"]
    g = 0
    for db in range(DK):
        ot = outp.tile([P, cap], BF16, tag="otT")
        for (to, ts) in _chunks(cap):
            pB = psB.tile([P, 512], F32, tag="pB2")
            for fi in range(FI):
                nc.tensor.matmul(
                    pB[:, :ts], (w2d[:, fi, db, :]),
                    (atiles[fi][:, to:to + ts]),
                    start=(fi == 0), stop=(fi == FI - 1))
            if g % 2 == 0:
                nc.scalar.activation(ot[:, to:to + ts], pB[:, :ts],
                                     AF.Identity)
            else:
                nc.vector.tensor_scalar(
                    out=ot[:, to:to + ts], in0=pB[:, :ts],
                    scalar1=0.0, scalar2=0.0, op0=OP.add, op1=OP.add)
            g += 1
        nc.scalar.dma_start(
            aps[pref + "out"][db * P:(db + 1) * P, :], ot[:])


def _emit_stage_b(tc, pools, pref, aps, cap, tiles, last=False):
    # b2 is added on the host; DVE applies the combine weight and writes
    # bf16 partials; the next batch's loads were already queued before
    # these stores, so sync-queue ordering cannot starve the PE. On the
    # final batch the trailing groups drain on ACT as well so the
    # end-of-program backlog clears twice as fast.
    nc = tc.nc
    wA, w2p, apool, hpool, outp, psA, psB = pools
    atiles, w2t = tiles["atiles"], tiles["w2t"]
    ballcw = tiles["ballcw"]
    CWO = 4 * FI
    ntt = (cap + P - 1) // P
    g = 0
    for tp in range(ntt):
        tn = min(P, cap - tp * P)
        tsl = slice(tp * P, tp * P + tn)
        ot = outp.tile([P, D], BF16, tag="ot")
        for dch in range(D // 512):
            dsl = slice(dch * 512, (dch + 1) * 512)
            pB = psB.tile([P, 512], F32, tag="pB2")
            for fi in range(FI):
                nc.tensor.matmul(
                    pB[:tn, :], (atiles[fi][:, tsl]), (w2t[:, fi, dsl]),
                    start=(fi == 0), stop=(fi == FI - 1))
            if last and g % 2 == 0:
                nc.scalar.activation(ot[:tn, dsl], pB[:tn, :], AF.Identity,
                                     scale=ballcw[:tn, CWO + tp:CWO + tp + 1])
            else:
                nc.vector.tensor_scalar_mul(
                    ot[:tn, dsl], pB[:tn, :],
                    ballcw[:tn, CWO + tp:CWO + tp + 1])
            g += 1
            if last and tp == ntt - 1:
                # split the final tile's store per 512-col half so the
                # first half streams out while the second is computed
                nc.scalar.dma_start(
                    aps[pref + "out"][tsl, dsl], ot[:tn, dsl])
        if not (last and tp == ntt - 1):
            # stores ride the scalar queue: the sync queue carries the
            # next batch's large weight/x prefetches; ordering behind
            # them would delay the output drain.
            nc.scalar.dma_start(aps[pref + "out"][tsl, :], ot[:tn, :])


def _build_sparse():
    nc = bacc.Bacc(
        "TRN2", target_bir_lowering=False, debug=False, num_devices=NCORES
    )
    aps = {}

    def inp(name, shape, dt=F32):
        aps[name] = nc.dram_tensor(name, shape, dt, kind="ExternalInput").ap()

    for pref, cap in (("a_", CAP_A), ("b_", CAP_B), ("s_", CAP_S)):
        ntt = (cap + P - 1) // P
        if pref == "a_":
            inp(pref + "xT", [D, cap], BF16)
            c1n = _chunks(cap)[1][1]
            inp(pref + "xc1e", [P, DK // 2, c1n], BF16)
            inp(pref + "xc1o", [P, DK // 2, c1n], BF16)
        else:
            inp(pref + "xe", [P, DK // 2, cap], BF16)
            inp(pref + "xo", [P, DK // 2, cap], BF16)
        inp(pref + "wcat", [FI, P, 4, DK, P], BF16)
        inp(pref + "ballcw", [P, 4 * FI + ntt])
        if pref == "a_":
            inp(pref + "w2T", [P, FI, DK, P], BF16)
            aps[pref + "out"] = nc.dram_tensor(
                pref + "out", [D, cap], BF16, kind="ExternalOutput").ap()
        else:
            inp(pref + "w2T", [P, FI, D], BF16)
            aps[pref + "out"] = nc.dram_tensor(
                pref + "out", [cap, D], BF16, kind="ExternalOutput").ap()
    for n in ("w1g", "w1l", "w3g", "w3l"):
        inp("a_" + n, [P, DK, P], BF16)

    tc_cls = _LeanTC if FAST_TEARDOWN else tile.TileContext
    with tc_cls(nc) as tc:
        with ExitStack() as ctx:
            wA = ctx.enter_context(tc.tile_pool(name="wA", bufs=3))
            w2p = ctx.enter_context(tc.tile_pool(name="w2p", bufs=2))
            apool = ctx.enter_context(tc.tile_pool(name="apool", bufs=1))
            hpool = ctx.enter_context(tc.tile_pool(name="hpool", bufs=4))
            outp = ctx.enter_context(tc.tile_pool(name="outp", bufs=6))
            psA = ctx.enter_context(
                tc.tile_pool(name="psA", bufs=2, space="PSUM"))
            psB = ctx.enter_context(
                tc.tile_pool(name="psB", bufs=4, space="PSUM"))
            pools = (wA, w2p, apool, hpool, outp, psA, psB)
            batches = (("a_", CAP_A), ("b_", CAP_B), ("s_", CAP_S))
            state, load = _mk_loader(tc, pools, aps)
            _emit_warmup(tc, pools)
            _emit_opening(tc, pools, aps, state, load, CAP_A)

            def L(kind, pref, *a):
                return lambda: load[kind](pref, *a)

            # Paced-prefetch schedule: position -> loads issued right
            # after that (fi, chunk) completes emission on the ACT queue.
            # Each load lands ~5-20us before its consumer with <=2
            # transfers in flight (the rings fair-share, so flooding
            # them delays the first-needed transfer).
            paces = {
                "a_": {
                    0: [L("wcat", "a_", 1)],
                    1: [L("wcat", "a_", 2)],
                    2: [L("wcat", "a_", 3)],
                    3: [L("w2T", "a_")],
                    4: [L("wcat", "b_", 0), L("xe", "b_", CAP_B)],
                    5: [L("xo", "b_", CAP_B), L("ballcw", "b_", CAP_B)],
                    6: [L("wcat", "b_", 1)],
                },
                "b_": {
                    0: [L("wcat", "b_", 2)],
                    1: [L("wcat", "b_", 3), L("w2T", "b_")],
                    2: [L("wcat", "s_", 0), L("xe", "s_", CAP_S)],
                    3: [L("xo", "s_", CAP_S), L("ballcw", "s_", CAP_S),
                        L("wcat", "s_", 1)],
                },
                "s_": {
                    0: [L("wcat", "s_", 2)],
                    1: [L("wcat", "s_", 3), L("w2T", "s_")],
                },
            }
            for i, (pref, cap) in enumerate(batches):
                _emit_stage_a(tc, pools, pref, aps, cap, state[pref],
                              first=(i == 0), pace=paces[pref])
                if pref == "a_":
                    _emit_stage_bT(tc, pools, pref, aps, cap, state[pref])
                else:
                    _emit_stage_b(tc, pools, pref, aps, cap, state[pref],
                                  last=(i == len(batches) - 1))
    nc.compile()
    return nc


# ---------------------------------------------------------------------------
# host-side prep
# ---------------------------------------------------------------------------

def _warr(w):      # [FH, D] -> [FI, P, DK, P] stage-A stationary layout
    return np.ascontiguousarray(
        w.T.reshape(DK, P, FI, P).transpose(2, 1, 0, 3))


def _bcol(b):      # [FH] -> [P, FI]
    return np.ascontiguousarray(b.reshape(FI, P).T)


def _gate(x, gate_w, gate_b):
    """Replicate the reference gate on CPU jax (bit-identical math)."""
    import jax
    import jax.numpy as jnp
    cpu = jax.devices("cpu")[0]
    with jax.default_device(cpu):
        xt = jnp.asarray(np.asarray(x, np.float32).reshape(T, D))
        logits = xt @ jnp.asarray(np.asarray(gate_w, np.float32)).T
        scores = jax.nn.softmax(logits.astype(jnp.float32), axis=-1)
        biased = scores + jnp.asarray(
            np.asarray(gate_b, np.float32)).astype(jnp.float32)
        idx = jax.lax.top_k(biased, 2)[1]
        weights = jnp.take_along_axis(scores, idx, axis=-1)
        return np.asarray(idx), np.asarray(weights)


def _prep_sparse(x, gate_w, gate_b, w1, b1, w3, b3, w2, b2,
                 sw1, sb1, sw3, sb3, sw2, sb2):
    f32 = np.float32
    xt = np.asarray(x, f32).reshape(T, D)
    xTq = np.ascontiguousarray(xt.T.astype(BF))     # [D, T] bf16

    idx, wts = _gate(x, gate_w, gate_b)             # [T, 2], [T, 2]
    toks = [[] for _ in range(E)]
    cws = [[] for _ in range(E)]
    for k in range(2):
        for t in range(T):
            e = int(idx[t, k])
            toks[e].append(t)
            cws[e].append(wts[t, k])
    counts = np.array([len(v) for v in toks])

    # pair heavy-with-light
    order = np.argsort(counts, kind='stable')
    eAs = [int(order[7 - g]) for g in range(NGROUPS)]   # heavier experts
    eBs = [int(order[g]) for g in range(NGROUPS)]       # lighter experts
    if counts[eAs].max() > CAP_A or counts[eBs].max() > CAP_B:
        return None, None, None  # fall back to dense

    def halves(w1e, b1e, w3e, b3e, w2e, b2e):
        """Per-F-half weight dict pieces for one expert's matrices."""
        w1e, w3e = np.asarray(w1e, f32), np.asarray(w3e, f32)
        b1e, b3e = np.asarray(b1e, f32), np.asarray(b3e, f32)
        w2e, b2e = np.asarray(w2e, f32), np.asarray(b2e, f32)
        out = []
        for h in range(2):
            fsl = slice(h * FH, (h + 1) * FH)
            parts = {
                "w1g": _warr(w1e[0::2][fsl].astype(BF)),
                "w1l": _warr(w1e[1::2][fsl].astype(BF)),
                "w3g": _warr(w3e[0::2][fsl].astype(BF)),
                "w3l": _warr(w3e[1::2][fsl].astype(BF)),
            }
            w2T = np.ascontiguousarray(
                (w2e.T[fsl] * (1.0 / ALPHA)).astype(BF))
            out.append({
                "wcat": np.ascontiguousarray(np.stack(
                    [parts[nm] for nm in WSETS], axis=2)),
                "w1g0": parts["w1g"][0], "w1l0": parts["w1l"][0],
                "w3g0": parts["w3g"][0], "w3l0": parts["w3l"][0],
                "ball": np.ascontiguousarray(np.concatenate(
                    [_bcol(b1e[0::2][fsl]), _bcol(b1e[1::2][fsl]),
                     _bcol(b3e[0::2][fsl]), _bcol(b3e[1::2][fsl])], axis=1)),
                "w2T": np.ascontiguousarray(
                    w2T.reshape(FI, P, D).transpose(1, 0, 2)),
                "w2TT": np.ascontiguousarray(
                    w2T.reshape(FI, P, DK, P).transpose(1, 0, 2, 3)),
            })
        return out

    def gather(tl, cwv, cap):
        n = len(tl)
        tpad = np.zeros(cap, np.int64)
        tpad[:n] = tl
        cpad = np.zeros(((cap + P - 1) // P) * P, f32)
        cpad[:n] = cwv
        xg = np.ascontiguousarray(xTq[:, tpad])
        cwcol = np.ascontiguousarray(
            cpad.reshape(-1, P).T)
        return xg, cwcol

    sh_halves = halves(sw1, sb1, sw3, sb3, sw2, sb2)

    def assemble(m, pref, hv, xpack, cwcol):
        m[pref + "wcat"] = hv["wcat"]
        m[pref + "w2T"] = hv["w2TT"] if pref == "a_" else hv["w2T"]
        m[pref + "ballcw"] = np.ascontiguousarray(
            np.concatenate([hv["ball"], cwcol], axis=1))
        if pref == "a_":
            m["a_xT"] = xpack
            c1o, c1n = _chunks(CAP_A)[1]
            x3 = xpack[:, c1o:c1o + c1n].reshape(DK, P, c1n)
            m["a_xc1e"] = np.ascontiguousarray(x3[0::2].transpose(1, 0, 2))
            m["a_xc1o"] = np.ascontiguousarray(x3[1::2].transpose(1, 0, 2))
            for nm in WSETS:
                m["a_" + nm] = hv[nm + "0"]
        else:
            m[pref + "xe"], m[pref + "xo"] = xpack

    def eo_split(xg, cap):
        x3 = xg.reshape(DK, P, cap)
        return (np.ascontiguousarray(x3[0::2].transpose(1, 0, 2)),
                np.ascontiguousarray(x3[1::2].transpose(1, 0, 2)))

    in_maps = [dict() for _ in range(NCORES)]
    meta = []
    for g in range(NGROUPS):
        eA, eB = eAs[g], eBs[g]
        xgA, cwA = gather(toks[eA], cws[eA], CAP_A)
        xgB, cwB = gather(toks[eB], cws[eB], CAP_B)
        ssl = slice(g * CAP_S, (g + 1) * CAP_S)
        xgS = np.ascontiguousarray(xTq[:, ssl])
        scw = np.ones((P, CAP_S // P), f32)
        meta.append((toks[eA], counts[eA], np.asarray(cws[eA], f32), eA,
                     toks[eB], counts[eB], np.asarray(cws[eB], f32), eB))
        hvA = halves(w1[eA], b1[eA], w3[eA], b3[eA], w2[eA], b2[eA])
        hvB = halves(w1[eB], b1[eB], w3[eB], b3[eB], w2[eB], b2[eB])
        for h in range(2):
            c = 2 * g + h
            m = in_maps[c]
            assemble(m, "a_", hvA[h], xgA, cwA)
            assemble(m, "b_", hvB[h], eo_split(xgB, CAP_B), cwB)
            assemble(m, "s_", sh_halves[h], eo_split(xgS, CAP_S), scw)
    return in_maps, meta, None


_PROGS = {}


def _get_program(kind):
    if kind not in _PROGS:
        _PROGS[kind] = {"sparse": _build_sparse, "dense": _build_dense}[kind]()
    return _PROGS[kind]


def kernel(x, gate_w, gate_b, w1, b1, w3, b3, w2, b2,
           sw1, sb1, sw3, sb3, sw2, sb2, _trace=False, _results=None,
           _force_dense=False):
    kw = {}
    if _trace:
        kw = dict(trace=True, trace_cores=list(range(NCORES)))
    args = (x, gate_w, gate_b, w1, b1, w3, b3, w2, b2,
            sw1, sb1, sw3, sb3, sw2, sb2)
    if not _force_dense:
        in_maps, meta, _ = _prep_sparse(*args)
    else:
        in_maps = None
    if in_maps is not None:
        nc = _get_program("sparse")
        res = run_bass_kernel_spmd(
            nc, in_maps, core_ids=list(range(NCORES)), **kw)
        if _results is not None:
            _results.append(res)
        f32 = np.float32
        out = np.zeros((T, D), f32)
        for g in range(NGROUPS):
            r0, r1 = res.results[2 * g], res.results[2 * g + 1]
            out[g * CAP_S:(g + 1) * CAP_S] = (
                r0["s_out"].astype(f32) + r1["s_out"].astype(f32))
        out += np.asarray(sb2, f32)          # shared-expert down bias
        for g in range(NGROUPS):
            r0, r1 = res.results[2 * g], res.results[2 * g + 1]
            tA, nA, cwA, eA, tB, nB, cwB, eB = meta[g]
            # a_out is the transposed raw yT [D, CAP_A]; apply cw here
            yTA = (r0["a_out"].astype(f32) + r1["a_out"].astype(f32))
            out[tA] += cwA[:nA, None] * (
                yTA.T[:nA] + np.asarray(b2[eA], f32)[None, :])
            out[tB] += (r0["b_out"][:nB].astype(f32)
                        + r1["b_out"][:nB].astype(f32)
                        + cwB[:nB, None] * np.asarray(b2[eB], f32))
        return out.reshape(np.asarray(x).shape).astype(np.float32)

    # dense fallback
    in_maps = _prep_dense(*args)
    nc = _get_program("dense")
    res = run_bass_kernel_spmd(nc, in_maps, core_ids=list(range(NCORES)), **kw)
    if _results is not None:
        _results.append(res)
    acc = np.zeros((T, D), np.float32)
    for c in range(NCORES):
        acc += res.results[c]["out"]
    return acc.reshape(np.asarray(x).shape).astype(np.float32)


# ---------------------------------------------------------------------------
# dense all-on-device fallback (V1): every core runs its expert over all
# tokens, masked by on-device gate weights; shared expert sharded on 2F.
# ---------------------------------------------------------------------------

TCH = 512
NTH = 2
TH = T // NTH
DKF = D // P
FIF = F // P


def _build_dense():
    nc = bacc.Bacc(
        "TRN2", target_bir_lowering=False, debug=False, num_devices=NCORES
    )
    aps = {}

    def inp(name, shape, dt=F32):
        aps[name] = nc.dram_tensor(name, shape, dt, kind="ExternalInput").ap()

    inp("xT", [D, T], F32R)
    inp("gw", [P, DKF * E], F32R)
    inp("gb", [P, E])
    inp("sel", [P, E])
    for n in ("w1g", "w1l", "w3g", "w3l"):
        inp(n, [FIF, P, DKF, P], F32R)
    for n in ("b1g", "b1l", "b3g", "b3l"):
        inp(n, [P, FIF + 1])
    inp("w2T", [F, D], F32R)
    inp("b2r", [1, D], F32R)
    for n in ("sw1g", "sw1l", "sw3g", "sw3l"):
        inp(n, [P, DKF, P], F32R)
    inp("sw2T", [P, D], F32R)
    inp("sb2r", [1, D], F32R)
    inp("ones", [1, P], F32R)
    aps["out"] = nc.dram_tensor("out", [T, D], F32, kind="ExternalOutput").ap()

    with tile.TileContext(nc) as tc:
        _emit_dense(tc, aps)
    nc.compile()
    return nc


def _emit_dense(tc, aps):
    nc = tc.nc
    ctx = ExitStack()

    with ctx:
        const = ctx.enter_context(tc.tile_pool(name="const", bufs=1))

        xsb = []
        for dk in range(DKF):
            t = const.tile([P, T], F32R, tag=f"x{dk}")
            nc.sync.dma_start(t[:], aps["xT"][dk * P:(dk + 1) * P, :])
            xsb.append(t)

        def load_const(name, shape, dt=F32):
            t = const.tile(shape, dt, tag=name)
            nc.sync.dma_start(t[:], aps[name][:])
            return t

        gw_sb = load_const("gw", [P, DKF * E], F32R)
        gb_sb = load_const("gb", [P, E])
        sel_sb = load_const("sel", [P, E])
        bcols = {n: load_const(n, [P, FIF + 1])
                 for n in ("b1g", "b1l", "b3g", "b3l")}
        b2r_sb = load_const("b2r", [1, D], F32R)
        sb2r_sb = load_const("sb2r", [1, D], F32R)
        sw2T_sb = load_const("sw2T", [P, D], F32R)
        ssw = {}
        for name in ("sw1g", "sw1l", "sw3g", "sw3l"):
            t = const.tile([P, DKF, P], F32R, tag=name)
            nc.sync.dma_start(t[:], aps[name][:])
            ssw[name] = t

        ones = const.tile([1, P], F32R, tag="ones")
        nc.sync.dma_start(ones[:], aps["ones"][:])
        ident = const.tile([E, E], F32, tag="ident")
        nc.vector.memset(ident[:], 0.0)
        from concourse.masks import make_identity
        make_identity(nc, ident[:], nomemset=True)

        cw = const.tile([P, T // P], F32, tag="cw")

        # ---- gate ----
        with tc.tile_pool(name="psG", bufs=2, space="PSUM") as psG, \
             tc.tile_pool(name="gtmp", bufs=1) as gtmp:
            NC = T // P
            logits_tb = const.tile([P, NC * E], F32, tag="logits_tb")
            logitsT = gtmp.tile([E, T], F32, tag="logitsT")
            for tch in range(T // TCH):
                pg = psG.tile([E, TCH], F32, tag="pslog")
                for dk in range(DKF):
                    nc.tensor.matmul(
                        pg[:],
                        (gw_sb[:, dk * E:(dk + 1) * E]),
                        (xsb[dk][:, tch * TCH:(tch + 1) * TCH]),
                        start=(dk == 0), stop=(dk == DKF - 1),
                    )
                nc.scalar.copy(logitsT[:, tch * TCH:(tch + 1) * TCH], pg[:])
            for j in range(NC):
                pt = psG.tile([P, E], F32, tag="pstr")
                nc.tensor.transpose(
                    pt[:], logitsT[:, j * P:(j + 1) * P], ident[:])
                nc.scalar.copy(logits_tb[:, j * E:(j + 1) * E], pt[:])

            eL = gtmp.tile([P, NC * E], F32, tag="eL")
            nc.scalar.activation(eL[:], logits_tb[:], AF.Exp)
            e3 = eL[:].rearrange("p (c e) -> p c e", e=E)
            ssum = gtmp.tile([P, NC], F32, tag="ssum")
            nc.vector.reduce_sum(ssum[:], e3, axis=mybir.AxisListType.X)
            rs = gtmp.tile([P, NC], F32, tag="rs")
            nc.vector.reciprocal(rs[:], ssum[:])
            scores = gtmp.tile([P, NC * E], F32, tag="scores")
            s3 = scores[:].rearrange("p (c e) -> p c e", e=E)
            nc.vector.tensor_mul(
                s3, e3, rs[:, :, None].broadcast_to((P, NC, E)))
            biased = gtmp.tile([P, NC * E], F32, tag="biased")
            bi3 = biased[:].rearrange("p (c e) -> p c e", e=E)
            nc.vector.tensor_add(
                bi3, s3, gb_sb[:, None, :].broadcast_to((P, NC, E)))
            m1 = gtmp.tile([P, NC], F32, tag="m1")
            nc.vector.reduce_max(m1[:], bi3, axis=mybir.AxisListType.X)
            mask1 = gtmp.tile([P, NC * E], F32, tag="mask1")
            mk3 = mask1[:].rearrange("p (c e) -> p c e", e=E)
            nc.vector.tensor_tensor(
                mk3, bi3, m1[:, :, None].broadcast_to((P, NC, E)), OP.is_ge)
            biased2 = gtmp.tile([P, NC * E], F32, tag="biased2")
            b23 = biased2[:].rearrange("p (c e) -> p c e", e=E)
            nc.vector.scalar_tensor_tensor(
                out=b23, in0=mk3, scalar=-1e30, in1=bi3,
                op0=OP.mult, op1=OP.add)
            m2 = gtmp.tile([P, NC], F32, tag="m2")
            nc.vector.reduce_max(m2[:], b23, axis=mybir.AxisListType.X)
            mask2 = gtmp.tile([P, NC * E], F32, tag="mask2")
            mq3 = mask2[:].rearrange("p (c e) -> p c e", e=E)
            nc.vector.tensor_tensor(
                mq3, bi3, m2[:, :, None].broadcast_to((P, NC, E)), OP.is_ge)
            cwf = gtmp.tile([P, NC * E], F32, tag="cwf")
            cf3 = cwf[:].rearrange("p (c e) -> p c e", e=E)
            nc.vector.tensor_mul(cf3, s3, mq3)
            nc.vector.tensor_mul(
                cf3, cf3, sel_sb[:, None, :].broadcast_to((P, NC, E)))
            nc.vector.reduce_sum(cw[:], cf3, axis=mybir.AxisListType.X)

        # ---- main ----
        wA = ctx.enter_context(tc.tile_pool(name="wA", bufs=3))
        w2p = ctx.enter_context(tc.tile_pool(name="w2p", bufs=3))
        apool = ctx.enter_context(tc.tile_pool(name="apool", bufs=1))
        hpool = ctx.enter_context(tc.tile_pool(name="hpool", bufs=2))
        outp = ctx.enter_context(tc.tile_pool(name="outp", bufs=3))
        psA = ctx.enter_context(tc.tile_pool(name="psA", bufs=2, space="PSUM"))
        psB = ctx.enter_context(tc.tile_pool(name="psB", bufs=2, space="PSUM"))
        psS = ctx.enter_context(tc.tile_pool(name="psS", bufs=2, space="PSUM"))

        afc = FIF + 1
        for th in range(NTH):
            tbase = th * TH
            atiles = []
            for fi in range(afc):
                at = apool.tile([P, TH], F32R, tag=f"a{fi}")
                atiles.append(at)
                if fi < FIF:
                    wt = {}
                    for nm in ("w1g", "w1l", "w3g", "w3l"):
                        t = wA.tile([P, DKF, P], F32R, tag=nm)
                        nc.sync.dma_start(t[:], aps[nm][fi])
                        wt[nm] = t
                    w_g1, w_l1 = wt["w1g"], wt["w1l"]
                    w_g3, w_l3 = wt["w3g"], wt["w3l"]
                else:
                    w_g1, w_l1 = ssw["sw1g"], ssw["sw1l"]
                    w_g3, w_l3 = ssw["sw3g"], ssw["sw3l"]
                bc1g = bcols["b1g"][:, fi:fi + 1]
                bc1l = bcols["b1l"][:, fi:fi + 1]
                bc3g = bcols["b3g"][:, fi:fi + 1]
                bc3l = bcols["b3l"][:, fi:fi + 1]

                for tt in range(TH // TCH):
                    tsl = slice(tt * TCH, (tt + 1) * TCH)
                    gsl = slice(tbase + tt * TCH, tbase + (tt + 1) * TCH)

                    def hpsum(wtile, ptag):
                        ps = psA.tile([P, TCH], F32, tag=ptag)
                        for dk in range(DKF):
                            nc.tensor.matmul(
                                ps[:], (wtile[:, dk, :]),
                                (xsb[dk][:, gsl]),
                                start=(dk == 0), stop=(dk == DKF - 1))
                        return ps

                    pg1 = hpsum(w_g1, "pA")
                    t1 = hpool.tile([P, TCH], F32, tag="tcp")
                    nc.scalar.activation(t1[:], pg1[:], AF.Identity, bias=bc1g)
                    pg3 = hpsum(w_g3, "pB")
                    hg = hpool.tile([P, TCH], F32, tag="hh")
                    nc.vector.scalar_tensor_tensor(
                        out=hg[:], in0=pg3[:], scalar=bc3g, in1=t1[:],
                        op0=OP.add, op1=OP.mult)
                    nc.vector.tensor_scalar_min(hg[:], hg[:], LIMIT)
                    gs = hpool.tile([P, TCH], F32, tag="gs")
                    nc.scalar.activation(gs[:], hg[:], AF.Silu, scale=ALPHA)

                    pl1 = hpsum(w_l1, "pA")
                    t2 = hpool.tile([P, TCH], F32, tag="tcp")
                    nc.scalar.activation(t2[:], pl1[:], AF.Identity, bias=bc1l)
                    pl3 = hpsum(w_l3, "pB")
                    hl = hpool.tile([P, TCH], F32, tag="hh")
                    nc.vector.scalar_tensor_tensor(
                        out=hl[:], in0=pl3[:], scalar=bc3l, in1=t2[:],
                        op0=OP.add, op1=OP.mult)
                    nc.vector.tensor_scalar(
                        out=hl[:], in0=hl[:], scalar1=LIMIT, scalar2=-LIMIT,
                        op0=OP.min, op1=OP.max)
                    nc.vector.tensor_scalar(
                        out=hl[:], in0=hl[:], scalar1=1.0 / ALPHA,
                        scalar2=1.0 / ALPHA, op0=OP.mult, op1=OP.add)
                    nc.vector.tensor_mul(atiles[fi][:, tsl], gs[:], hl[:])

            for tp in range(TH // P):
                j = th * (TH // P) + tp
                tsl = slice(tp * P, (tp + 1) * P)
                for dch in range(D // TCH):
                    dsl = slice(dch * TCH, (dch + 1) * TCH)
                    pB = psB.tile([P, TCH], F32, tag="pB2")
                    nc.tensor.matmul(pB[:], (ones[:]),
                                     (b2r_sb[0:1, dsl]),
                                     start=True, stop=False)
                    for fi in range(FIF):
                        wt2 = w2p.tile([P, TCH], F32R, tag="w2t")
                        nc.sync.dma_start(
                            wt2[:], aps["w2T"][fi * P:(fi + 1) * P, dsl])
                        nc.tensor.matmul(
                            pB[:], (atiles[fi][:, tsl]), (wt2[:]),
                            start=False, stop=(fi == FIF - 1))
                    pS = psS.tile([P, TCH], F32, tag="pS")
                    nc.tensor.matmul(pS[:], (ones[:]),
                                     (sb2r_sb[0:1, dsl]),
                                     start=True, stop=False)
                    nc.tensor.matmul(
                        pS[:], (atiles[FIF][:, tsl]), (sw2T_sb[:, dsl]),
                        start=False, stop=True)
                    ot = outp.tile([P, TCH], F32, tag="ot")
                    nc.vector.tensor_scalar_mul(ot[:], pB[:], cw[:, j:j + 1])
                    nc.vector.tensor_add(ot[:], pS[:], ot[:])
                    nc.sync.dma_start(
                        aps["out"][tbase + tp * P:tbase + (tp + 1) * P, dsl],
                        ot[:])


def _prep_dense(x, gate_w, gate_b, w1, b1, w3, b3, w2, b2,
                sw1, sb1, sw3, sb3, sw2, sb2):
    f32 = np.float32
    xt = np.asarray(x, f32).reshape(T, D)
    xT = np.ascontiguousarray(xt.T)
    gwT = np.asarray(gate_w, f32).T
    gw_sb = np.ascontiguousarray(
        gwT.reshape(DKF, P, E).transpose(1, 0, 2).reshape(P, DKF * E))
    gb_bc = np.ascontiguousarray(
        np.broadcast_to(np.asarray(gate_b, f32), (P, E)))

    sw1 = np.asarray(sw1, f32)
    sw3 = np.asarray(sw3, f32)
    sw2T = np.asarray(sw2, f32).T
    sb1 = np.asarray(sb1, f32)
    sb3 = np.asarray(sb3, f32)
    sb2 = np.asarray(sb2, f32)

    def fwarr(w):      # [F, D] -> [FIF, P, DKF, P]
        return np.ascontiguousarray(
            w.T.reshape(DKF, P, FIF, P).transpose(2, 1, 0, 3))

    def swarr(w_sl):
        return np.ascontiguousarray(
            w_sl.T.reshape(DKF, P, P).transpose(1, 0, 2))

    def bcol2(b, sb_sl):
        return np.ascontiguousarray(
            np.concatenate([b.reshape(FIF, P).T, sb_sl[:, None]], axis=1))

    in_maps = []
    for c in range(NCORES):
        sel = np.zeros((P, E), f32)
        sel[:, c] = 1.0
        w1c = np.asarray(w1[c], f32)
        w3c = np.asarray(w3[c], f32)
        b1c = np.asarray(b1[c], f32)
        b3c = np.asarray(b3[c], f32)
        fsl = slice(c * P, (c + 1) * P)
        m = {
            "xT": xT, "gw": gw_sb, "gb": gb_bc, "sel": sel,
            "w1g": fwarr(w1c[0::2]), "w1l": fwarr(w1c[1::2]),
            "w3g": fwarr(w3c[0::2]), "w3l": fwarr(w3c[1::2]),
            "b1g": bcol2(b1c[0::2], sb1[0::2][fsl]),
            "b1l": bcol2(b1c[1::2], sb1[1::2][fsl]),
            "b3g": bcol2(b3c[0::2], sb3[0::2][fsl]),
            "b3l": bcol2(b3c[1::2], sb3[1::2][fsl]),
            "w2T": np.ascontiguousarray(np.asarray(w2[c], f32).T),
            "b2r": np.asarray(b2[c], f32)[None, :],
            "sw1g": swarr(sw1[0::2][fsl]), "sw1l": swarr(sw1[1::2][fsl]),
            "sw3g": swarr(sw3[0::2][fsl]), "sw3l": swarr(sw3[1::2][fsl]),
            "sw2T": np.ascontiguousarray(sw2T[fsl]),
            "sb2r": (sb2 if c == 0 else np.zeros_like(sb2))[None, :],
            "ones": np.ones((1, P), f32),
        }
        in_maps.append(m)
    return in_maps


if __name__ == "__main__":
    rng = np.random.RandomState(0)
    sd = 1 / 32.0
    ins = {
        "x": rng.randn(2, 1024, 1024).astype(np.float32),
        "gate_w": (rng.randn(E, D) * sd).astype(np.float32),
        "gate_b": (rng.randn(E) * 0.01).astype(np.float32),
        "w1": (rng.randn(E, 2 * F, D) * sd).astype(np.float32),
        "b1": (rng.randn(E, 2 * F) * 0.01).astype(np.float32),
        "w3": (rng.randn(E, 2 * F, D) * sd).astype(np.float32),
        "b3": (rng.randn(E, 2 * F) * 0.01).astype(np.float32),
        "w2": (rng.randn(E, D, F) * sd).astype(np.float32),
        "b2": (rng.randn(E, D) * 0.01).astype(np.float32),
        "sw1": (rng.randn(2 * F, D) * sd).astype(np.float32),
        "sb1": (rng.randn(2 * F) * 0.01).astype(np.float32),
        "sw3": (rng.randn(2 * F, D) * sd).astype(np.float32),
        "sb3": (rng.randn(2 * F) * 0.01).astype(np.float32),
        "sw2": (rng.randn(D, F) * sd).astype(np.float32),
        "sb2": (rng.randn(D) * 0.01).astype(np.float32),
    }
    out = kernel(**ins)
    print("OK", out.shape, out.dtype, np.abs(out).mean())

